# revision 42
# baseline (speedup 1.0000x reference)
"""2-layer GAT (PyG GATConv, concat=False, self-loops) on 8 Trainium2 cores.

Design: nodes/edges partitioned by destination across 8 cores; each core
owns a 6250-dst range, dsts sorted by (degree//4, must-lo count) into 49
blocks of 128 PSUM lanes.  Per-edge source data is fetched with dma_gather
from a 512B-per-node table in HBM; each all-bf16 row is the raw psum of
one augmented matmul x @ [W | W@a_src | W@a_dst]: [h x128 | asrc x4 |
adst x4 | pad].  The gather is SWDGE descriptor-emission bound (~8ns/row
on the Q7), so row bytes are nearly free while row COUNT matters: padding
is minimized by a per-lane-balanced split of edges between two overlapping
int16-indexable views (lo=[0,32767), hi=[17241,50008) of the 50008-row
g-space table - 8 slabs of 6251 rows, one junk/sentinel row per slab), by
excluding self-loop edges (each core streams its own dst rows sequentially,
which also provides a_dst and the self contribution), and by one shared
index stream for both layers.  Padding slots point at the sentinel row
(asrc=-350 => exp(LRelu(...)) ~ 0, h=0).  Scatter is free: slot columns
accumulate into per-block PSUM via identity-weight matmuls, 3 columns per
matmul.  Gather calls batch 15 columns (two calls fit the enlarged 64KB
SWDGE descriptor ring, overlapping emission with drain; multi-packet mode
lifts the 1008-index single-packet limit).  Layer boundary: the layer-1
epilogue emits a transposed bf16 output in two column halves; each half is
AllGathered separately (the first mid-layer) and a replicated stage-A
rebuilds the layer-2 table, overlapping layer-1's tail.  Per-block dst
reads (D-phase) are precomputed during idle stage-A windows.
"""
import sys
sys.path.insert(0, "/opt/trn_rl_repo")

import numpy as np
import ml_dtypes

import concourse.bass as bass
import concourse.bacc as bacc
import concourse.mybir as mybir
from concourse.bass_utils import run_bass_kernel_spmd
from concourse.tile import TileContext

N = 50000
E = 1600000
IN = 128
H = 4
F = 32
NEG = 0.2
NCORES = 8
PERC = N // NCORES           # 6250
SLAB = PERC + 1              # 6251 (junk row at pos 6250)
TROWS = NCORES * SLAB        # 50008
NBLK = (PERC + 127) // 128   # 49
LO_END = 32767               # lo view = rows [0, 32767)
HI_START = TROWS - 32767     # 17241; hi view = rows [17241, 50008)
SENT_ROW = 3 * SLAB + PERC   # 25003 (core 3's junk row; inside the overlap)
RW = 256                     # table row: 256 bf16 slots = 512B:
                             # [h x128 | asrc x4 | adst x4 | 120 pad]
MAXC = int(__import__('os').environ.get('GAT_MAXC', '15'))  # max gather columns per dma_gather call
GRP = 3                      # aggregation matmul: 3 slot-columns per matmul


def _chunks(n):
    return [min(MAXC, n - s) for s in range(0, n, MAXC)]


def _pack_idx(idx_flat):
    """[n] -> [128, n/16] int16; idx i -> (partition i%16, col i//16), x8."""
    n = idx_flat.shape[0]
    assert n % 16 == 0
    a = idx_flat.reshape(n // 16, 16).T.astype(np.int16)
    return np.ascontiguousarray(np.tile(a, (8, 1)))


def _preprocess(edge_index):
    """Index preprocessing. Returns per-core gidx + shared structure."""
    src0 = edge_index[0].astype(np.int64)
    dst0 = edge_index[1].astype(np.int64)
    deg = np.bincount(dst0, minlength=N)          # gather-degree (no self)

    def build_perms(lo_cnt):
        perms = []
        pos_of = np.empty(N, np.int64)
        for c in range(NCORES):
            ids = np.arange(c * PERC, (c + 1) * PERC)
            order = np.lexsort((-lo_cnt[ids], -(deg[ids] // 4)))
            perm = ids[order]
            perms.append(perm)
            pos_of[perm] = np.arange(PERC)
        return perms, pos_of

    # pass 1: degree only -> provisional rows -> per-dst must-lo counts
    perms, pos_of = build_perms(np.zeros(N, np.int64))
    for _ in range(2):
        row_of = (np.arange(N) // PERC) * SLAB + pos_of
        must_lo = row_of[src0] < HI_START
        lo_cnt = np.bincount(dst0[must_lo], minlength=N)
        perms, pos_of = build_perms(lo_cnt)

    row_of = (np.arange(N) // PERC) * SLAB + pos_of
    r_src = row_of[src0]
    kind = np.where(r_src < HI_START, 0, np.where(r_src >= LO_END, 1, 2))

    lane_all = pos_of[dst0]
    core_all = dst0 // PERC
    blk_all = lane_all // 128

    # per-block slot counts nl/nh (cross-core maxima)
    A = np.zeros(NBLK, np.int64)
    B = np.zeros(NBLK, np.int64)
    C = np.zeros(NBLK, np.int64)
    cnt_l = np.zeros((NCORES, PERC), np.int64)
    cnt_h = np.zeros((NCORES, PERC), np.int64)
    cnt_f = np.zeros((NCORES, PERC), np.int64)
    for c in range(NCORES):
        m = core_all == c
        lane = lane_all[m]
        k = kind[m]
        cnt_l[c] = np.bincount(lane[k == 0], minlength=PERC)
        cnt_h[c] = np.bincount(lane[k == 1], minlength=PERC)
        cnt_f[c] = np.bincount(lane[k == 2], minlength=PERC)
    for b in range(NBLK):
        sl = slice(b * 128, min((b + 1) * 128, PERC))
        A[b] = cnt_l[:, sl].max()
        B[b] = cnt_h[:, sl].max()
        C[b] = (cnt_l[:, sl] + cnt_h[:, sl] + cnt_f[:, sl]).max()
    nl = A.copy()
    nh = np.maximum(B, C - A)
    S = int((nl + nh).sum())
    col_lo = np.zeros(NBLK, np.int64)   # column start of block's lo run
    col_hi = np.zeros(NBLK, np.int64)
    col = 0
    for b in range(NBLK):
        col_lo[b] = col
        col += nl[b]
        col_hi[b] = col
        col += nh[b]
    assert col == S

    # per-core column fill
    gidx = []
    sent_lo, sent_hi = SENT_ROW, SENT_ROW - HI_START
    for c in range(NCORES):
        m = core_all == c
        lane = lane_all[m]
        k = kind[m].copy()
        rows = r_src[m]
        # flex -> lo for the first a_i of each lane, rest -> hi
        nh_of_lane = nh[np.arange(PERC) // 128]
        a_i = np.maximum(0, cnt_h[c] + cnt_f[c] - nh_of_lane)
        # order edges by (lane, kind-rank, row); flex edges get rank by
        # position so the first a_i go to lo
        order = np.lexsort((rows, k, lane))
        lane_s, k_s, rows_s = lane[order], k[order], rows[order]
        # cumcount within (lane, kind)
        start = np.r_[True, (lane_s[1:] != lane_s[:-1]) | (k_s[1:] != k_s[:-1])]
        grp_id = np.cumsum(start) - 1
        first_pos = np.full(grp_id[-1] + 1, 1 << 62, np.int64)
        np.minimum.at(first_pos, grp_id, np.arange(len(grp_id)))
        cum = np.arange(len(grp_id)) - first_pos[grp_id]
        # stream + slot per edge
        is_lo = (k_s == 0) | ((k_s == 2) & (cum < a_i[lane_s]))
        slot = np.where(
            k_s == 0, cum,                                   # must-lo
            np.where(k_s == 1, cum,                          # must-hi
                     np.where(is_lo, cnt_l[c][lane_s] + cum,  # flex->lo
                              cnt_h[c][lane_s] + cum - a_i[lane_s])))  # flex->hi
        b_s = lane_s // 128
        colidx = np.where(is_lo, col_lo[b_s] + slot, col_hi[b_s] + slot)
        grid = np.full((S, 128), -1, np.int64)
        grid[colidx, lane_s % 128] = rows_s
        # sentinels + view-relative index
        lo_cols = np.zeros(S, bool)
        for b in range(NBLK):
            lo_cols[col_lo[b]:col_lo[b] + nl[b]] = True
        grid[lo_cols] = np.where(grid[lo_cols] < 0, sent_lo, grid[lo_cols])
        grid[~lo_cols] = np.where(grid[~lo_cols] < 0, sent_hi + HI_START,
                                  grid[~lo_cols]) - HI_START
        assert grid.min() >= 0 and grid.max() < LO_END
        gidx.append(_pack_idx(grid.reshape(-1)))

    return dict(gidx=gidx, nl=nl, nh=nh, S=S, col_lo=col_lo, col_hi=col_hi,
                perms=perms, pos_of=pos_of, row_of=row_of)


SLA = 8 * 128   # stage-A slab width


def _stage_a(nc, pools, views, w_sb, tbl, kdim):
    """[h | asrc | adst] = lhsT.T @ Waug -> rows packed into tbl.

    views: list of (lhsT_view [kdim, W], row0); W <= SLA.  Row layout
    (all bf16, matching the augmented-matmul psum column order):
    [h x128 | asrc x4 | adst x4 | 120 slots garbage].
    """
    sb, ps = pools
    f32, bf16 = mybir.dt.float32, mybir.dt.bfloat16
    for (view, row0) in views:
        Wt = view.shape[1]
        nt = (Wt + 127) // 128
        xsb = sb.tile([kdim, SLA], bf16, tag="xa")
        nc.sync.dma_start(out=xsb[:, 0:Wt], in_=view)
        stg = sb.tile([128, SLA // 128, RW], bf16, tag="sa")
        for t in range(nt):
            w = min(128, Wt - t * 128)
            psum = ps.tile([128, 136], f32, tag="pa")
            nc.tensor.matmul(out=psum[0:w, :], lhsT=xsb[:, t * 128:t * 128 + w],
                             rhs=w_sb[:], start=True, stop=True)
            nc.vector.tensor_copy(out=stg[0:w, t, 0:136], in_=psum[0:w, 0:136])
        nfull = Wt // 128
        if nfull:
            nc.sync.dma_start(
                out=tbl[row0:row0 + nfull * 128, :].rearrange(
                    "(t p) c -> p t c", p=128),
                in_=stg[:, 0:nfull, :])
        if Wt % 128:
            nc.sync.dma_start(out=tbl[row0 + nfull * 128:row0 + Wt, :],
                              in_=stg[0:Wt % 128, nfull, :])


def _build_program(meta):
    nl, nh, S = meta["nl"], meta["nh"], meta["S"]
    import os as _os
    NSWQ = int(_os.environ.get('GAT_NSWQ', '1'))
    nc = bacc.Bacc("TRN2", target_bir_lowering=False, debug=False,
                   num_devices=NCORES, num_swdge_queues=NSWQ,
                   dynamic_dma_scratch_size=int(_os.environ.get('GAT_SCRATCH', '65536')))

    f32, bf16, i16 = mybir.dt.float32, mybir.dt.bfloat16, mybir.dt.int16
    xTg = nc.dram_tensor("xTg", [IN, TROWS], bf16, kind="ExternalInput")
    xTs = nc.dram_tensor("xTs", [IN, PERC], bf16, kind="ExternalInput")
    w1 = nc.dram_tensor("w1", [IN, 136], bf16, kind="ExternalInput")
    w2 = nc.dram_tensor("w2", [F, 136], bf16, kind="ExternalInput")
    gidx = nc.dram_tensor("gidx", [128, S * 8], i16, kind="ExternalInput")
    identb = nc.dram_tensor("identb", [128, 128], bf16, kind="ExternalInput")
    identf = nc.dram_tensor("identf", [128, 128], f32, kind="ExternalInput")
    sent1 = nc.dram_tensor("sent1", [1, RW], bf16, kind="ExternalInput")
    sent2 = nc.dram_tensor("sent2", [1, RW], bf16, kind="ExternalInput")
    b1r = nc.dram_tensor("b1r", [128, F], f32, kind="ExternalInput")
    b2r = nc.dram_tensor("b2r", [128, F], f32, kind="ExternalInput")

    T1 = nc.dram_tensor("T1", [TROWS, RW], bf16)
    T2 = nc.dram_tensor("T2", [TROWS, RW], bf16)
    Ts1 = nc.dram_tensor("Ts1", [PERC, RW], bf16)
    Ts2 = nc.dram_tensor("Ts2", [PERC, RW], bf16)
    SPLIT = 24 * 128             # o1T column split (block 24 boundary)
    o1Ta = nc.dram_tensor("o1Ta", [F, SPLIT], bf16)
    o1Tb = nc.dram_tensor("o1Tb", [F, PERC - SPLIT], bf16)
    o1Tga = nc.dram_tensor("o1Tga", [NCORES * F, SPLIT], bf16,
                           addr_space="Shared")
    o1Tgb = nc.dram_tensor("o1Tgb", [NCORES * F, PERC - SPLIT], bf16,
                           addr_space="Shared")
    out2p = nc.dram_tensor("out2p", [PERC, F], f32, kind="ExternalOutput")

    NC3 = ((MAXC + GRP - 1) // GRP) * GRP  # rhs tile columns (33)

    with TileContext(nc) as tc:
        with (
            tc.tile_pool(name="cons", bufs=1) as cons,
            tc.tile_pool(name="sbA", bufs=3) as sbA,
            tc.tile_pool(name="psA", bufs=2, space="PSUM") as psA,
            tc.tile_pool(name="dp", bufs=2) as dp,
            tc.tile_pool(name="gp", bufs=5) as gp,
            tc.tile_pool(name="rp", bufs=4) as rp,
            tc.tile_pool(name="ep", bufs=8) as ep,
            tc.tile_pool(name="pp", bufs=1) as pp,
            tc.tile_pool(name="psE", bufs=3, space="PSUM") as psE,
            tc.tile_pool(name="psT", bufs=1, space="PSUM") as psT,
        ):
            identb_sb = cons.tile([128, 128], bf16)
            nc.sync.dma_start(out=identb_sb[:], in_=identb[:, :])
            identf_sb = cons.tile([128, 128], f32)
            nc.sync.dma_start(out=identf_sb[:], in_=identf[:, :])
            w1_sb = cons.tile([IN, 136], bf16)
            nc.sync.dma_start(out=w1_sb[:], in_=w1[:, :])
            w2_sb = cons.tile([F, 136], bf16)
            nc.sync.dma_start(out=w2_sb[:], in_=w2[:, :])
            b1r_sb = cons.tile([128, F], f32)
            nc.sync.dma_start(out=b1r_sb[:], in_=b1r[:, :])
            b2r_sb = cons.tile([128, F], f32)
            nc.sync.dma_start(out=b2r_sb[:], in_=b2r[:, :])
            sent1_sb = cons.tile([1, RW], bf16)
            nc.sync.dma_start(out=sent1_sb[:], in_=sent1[:, :])
            sent2_sb = cons.tile([1, RW], bf16)
            nc.sync.dma_start(out=sent2_sb[:], in_=sent2[:, :])
            gidx_sb = cons.tile([128, S * 8], i16)
            nc.sync.dma_start(out=gidx_sb[:], in_=gidx[:, :])

            def d_phase(tself):
                """Per-block dst-slab read; returns pre[b] = (adl, rhs_s)
                tiles that persist until the g-phase."""
                pre = {}
                for b in range(NBLK):
                    w_b = min(128, PERC - b * 128)
                    dt = dp.tile([128, RW], bf16, tag="dt")
                    if w_b < 128:
                        nc.vector.memset(dt[:], 0.0)
                    nc.sync.dma_start(out=dt[0:w_b, :],
                                      in_=tself[b * 128:b * 128 + w_b, :])
                    dhb = dt[:, 0:128]                 # [128, 128] h-major
                    adl = pp.tile([128, H], bf16, tag=f"adl{b}")
                    nc.vector.tensor_copy(out=adl[:], in_=dt[:, 132:136])
                    # self edge: e = LRelu(asrc + adst); w = exp(e)
                    es = ep.tile([128, H], f32, tag="es")
                    nc.vector.tensor_tensor(out=es[:], in0=dt[:, 128:132],
                                            in1=dt[:, 132:136],
                                            op=mybir.AluOpType.add)
                    es2 = ep.tile([128, H], f32, tag="es2")
                    nc.vector.tensor_scalar(out=es2[:], in0=es[:], scalar1=NEG,
                                            scalar2=None, op0=mybir.AluOpType.mult)
                    nc.vector.tensor_tensor(out=es2[:], in0=es2[:], in1=es[:],
                                            op=mybir.AluOpType.max)
                    wsb = ep.tile([128, H], bf16, tag="wsb")
                    nc.scalar.activation(out=wsb[:], in_=es2[:],
                                         func=mybir.ActivationFunctionType.Exp)
                    rhs_s = pp.tile([128, 4 + H * F], f32, tag=f"rs{b}")
                    nc.vector.tensor_copy(out=rhs_s[:, 0:4], in_=wsb[:])
                    nc.vector.tensor_tensor(
                        out=rhs_s[:, 4:132].rearrange("p (h f) -> p h f", f=F),
                        in0=dhb.rearrange("p (h f) -> p h f", f=F),
                        in1=wsb[:].unsqueeze(2).to_broadcast([128, H, F]),
                        op=mybir.AluOpType.mult)
                    pre[b] = (adl, rhs_s)
                return pre

            qctr = [0]

            def edge_layer(tbl, pre, bias_sb, is_layer1):
                tbl_lo = tbl[0:LO_END, :]
                tbl_hi = tbl[HI_START:TROWS, :]
                for b in range(NBLK):
                    w_b = min(128, PERC - b * 128)
                    adl, rhs_s = pre[b]

                    # ---- gathered slots ----
                    psum = psE.tile([128, GRP * 132], f32, tag="acc")
                    n_tri = sum((cc + GRP - 1) // GRP
                                for nn in (int(nl[b]), int(nh[b]))
                                for cc in _chunks(nn))
                    tri = 0
                    for half in range(2):
                        ncols_all = int(nl[b]) if half == 0 else int(nh[b])
                        col0 = int(meta["col_lo"][b]) if half == 0 \
                            else int(meta["col_hi"][b])
                        view = tbl_lo if half == 0 else tbl_hi
                        for s0 in range(0, ncols_all, MAXC):
                            ncc = min(MAXC, ncols_all - s0)
                            nc3 = ((ncc + GRP - 1) // GRP) * GRP
                            g = gp.tile([128, MAXC, RW], bf16, tag="g")
                            nc.gpsimd.dma_gather(
                                g[:, 0:ncc, :], view,
                                gidx_sb[:, (col0 + s0) * 8:(col0 + s0 + ncc) * 8],
                                ncc * 128, ncc * 128, RW,
                                single_packet=(ncc * 128 <= 1008),
                                queue_num=qctr[0] % NSWQ)
                            qctr[0] += 1
                            gh = g[:, 0:ncc, 0:128]               # [*,nc,128]
                            al = ep.tile([128, MAXC, H], f32, tag="al")
                            nc.vector.tensor_tensor(
                                out=al[:, 0:ncc, :], in0=g[:, 0:ncc, 128:132],
                                in1=adl[:].unsqueeze(1).to_broadcast(
                                    [128, ncc, H]),
                                op=mybir.AluOpType.add)
                            alf = al[:, 0:ncc, :].rearrange("p n h -> p (n h)")
                            e2 = ep.tile([128, MAXC, H], f32, tag="e2")
                            e2f = e2[:, 0:ncc, :].rearrange("p n h -> p (n h)")
                            nc.vector.tensor_scalar(
                                out=e2f, in0=alf, scalar1=NEG, scalar2=None,
                                op0=mybir.AluOpType.mult)
                            nc.vector.tensor_tensor(
                                out=e2f, in0=e2f, in1=alf,
                                op=mybir.AluOpType.max)
                            rhs = rp.tile([128, NC3, 132], bf16, tag="rhs")
                            if nc3 > ncc:
                                nc.vector.memset(rhs[:, ncc:nc3, :], 0.0)
                            nc.scalar.activation(
                                out=rhs[:, 0:ncc, 0:4], in_=e2[:, 0:ncc, :],
                                func=mybir.ActivationFunctionType.Exp)
                            nc.vector.tensor_tensor(
                                out=rhs[:, 0:ncc, 4:132].rearrange(
                                    "p n (h f) -> p n h f", f=F),
                                in0=gh.rearrange(
                                    "p n (h f) -> p n h f", f=F),
                                in1=rhs[:, 0:ncc, 0:4].unsqueeze(3).to_broadcast(
                                    [128, ncc, H, F]),
                                op=mybir.AluOpType.mult)
                            for t in range(nc3 // GRP):
                                nc.tensor.matmul(
                                    out=psum[:],
                                    lhsT=identb_sb[:],
                                    rhs=rhs[:, t * GRP:(t + 1) * GRP, :].rearrange(
                                        "p a b -> p (a b)"),
                                    start=(tri == 0), stop=(tri == n_tri - 1))
                                tri += 1
                    assert tri == n_tri

                    # ---- epilogue ----
                    U = ep.tile([128, 132], f32, tag="U")
                    nc.vector.tensor_tensor(out=U[:], in0=rhs_s[:],
                                            in1=psum[:, 0:132],
                                            op=mybir.AluOpType.add)
                    nc.vector.tensor_tensor(out=U[:], in0=U[:],
                                            in1=psum[:, 132:264],
                                            op=mybir.AluOpType.add)
                    nc.vector.tensor_tensor(out=U[:], in0=U[:],
                                            in1=psum[:, 264:396],
                                            op=mybir.AluOpType.add)
                    sden = ep.tile([128, H], f32, tag="sden")
                    nc.vector.tensor_scalar(out=sden[:], in0=U[:, 0:4],
                                            scalar1=1e-16, scalar2=None,
                                            op0=mybir.AluOpType.add)
                    rv = ep.tile([128, H], f32, tag="rv")
                    nc.vector.reciprocal(out=rv[:], in_=sden[:])
                    nc.vector.tensor_scalar(out=rv[:], in0=rv[:], scalar1=1.0 / H,
                                            scalar2=None,
                                            op0=mybir.AluOpType.mult)
                    m = ep.tile([128, H * F], f32, tag="m")
                    nc.vector.tensor_tensor(
                        out=m[:].rearrange("p (h f) -> p h f", f=F),
                        in0=U[:, 4:132].rearrange("p (h f) -> p h f", f=F),
                        in1=rv[:].unsqueeze(2).to_broadcast([128, H, F]),
                        op=mybir.AluOpType.mult)
                    o = ep.tile([128, F], f32, tag="o")
                    nc.vector.tensor_tensor(out=o[:], in0=m[:, 0:F],
                                            in1=m[:, F:2 * F],
                                            op=mybir.AluOpType.add)
                    o2 = ep.tile([128, F], f32, tag="o2t")
                    nc.vector.tensor_tensor(out=o2[:], in0=m[:, 2 * F:3 * F],
                                            in1=m[:, 3 * F:4 * F],
                                            op=mybir.AluOpType.add)
                    nc.vector.tensor_tensor(out=o[:], in0=o[:], in1=o2[:],
                                            op=mybir.AluOpType.add)
                    nc.vector.tensor_tensor(out=o[:], in0=o[:], in1=bias_sb[:],
                                            op=mybir.AluOpType.add)
                    if is_layer1:
                        # ELU
                        m0 = ep.tile([128, F], f32, tag="m0")
                        nc.vector.tensor_scalar(out=m0[:], in0=o[:], scalar1=0.0,
                                                scalar2=None,
                                                op0=mybir.AluOpType.min)
                        em = ep.tile([128, F], f32, tag="em")
                        nc.scalar.activation(out=em[:], in_=m0[:],
                                             func=mybir.ActivationFunctionType.Exp)
                        nc.vector.tensor_scalar(out=em[:], in0=em[:], scalar1=-1.0,
                                                scalar2=None,
                                                op0=mybir.AluOpType.add)
                        nc.vector.tensor_tensor(out=o[:], in0=o[:], in1=em[:],
                                                op=mybir.AluOpType.max)
                        # transpose -> o1T (bf16) + h2 = o1 @ W2 -> Ts2
                        pT = psT.tile([F, 128], f32, tag="pT")
                        nc.tensor.transpose(out=pT[:], in_=o[:],
                                            identity=identf_sb[:])
                        oT = ep.tile([F, 128], bf16, tag="oT")
                        nc.vector.tensor_copy(out=oT[:], in_=pT[:])
                        if b * 128 < SPLIT:
                            nc.sync.dma_start(
                                out=o1Ta[:, b * 128:b * 128 + w_b],
                                in_=oT[:, 0:w_b])
                        else:
                            nc.sync.dma_start(
                                out=o1Tb[:, b * 128 - SPLIT:
                                         b * 128 - SPLIT + w_b],
                                in_=oT[:, 0:w_b])
                        if b == SPLIT // 128 - 1:
                            # first-half allgather + stage-A2 overlap the
                            # remaining layer-1 blocks
                            nc.gpsimd.collective_compute(
                                "AllGather", mybir.AluOpType.bypass,
                                replica_groups=[list(range(NCORES))],
                                ins=[o1Ta[:].opt()], outs=[o1Tga[:].opt()])
                            viewsA = []
                            for r in range(NCORES):
                                for p0 in range(0, SPLIT, SLA):
                                    viewsA.append(
                                        (o1Tga[r * F:(r + 1) * F,
                                               p0:min(p0 + SLA, SPLIT)],
                                         r * SLAB + p0))
                            _stage_a(nc, (sbA, psA), viewsA, w2_sb, T2, F)
                        ps2 = psT.tile([128, 136], f32, tag="ps2")
                        nc.tensor.matmul(out=ps2[:], lhsT=oT[:], rhs=w2_sb[:],
                                         start=True, stop=True)
                        st2 = ep.tile([128, RW], bf16, tag="st2")
                        nc.vector.tensor_copy(out=st2[:, 0:136],
                                              in_=ps2[:, 0:136])
                        nc.sync.dma_start(out=Ts2[b * 128:b * 128 + w_b, :],
                                          in_=st2[0:w_b, :])
                    else:
                        nc.sync.dma_start(out=out2p[b * 128:b * 128 + w_b, :],
                                          in_=o[0:w_b, :])

            # ---- stage A, layer 1: self slab first, then full table ----
            viewsS = [(xTs[:, s0:min(s0 + SLA, PERC)], s0)
                      for s0 in range(0, PERC, SLA)]
            _stage_a(nc, (sbA, psA), viewsS, w1_sb, Ts1, IN)
            views1 = [(xTg[:, s0:min(s0 + SLA, TROWS)], s0)
                      for s0 in range(0, TROWS, SLA)]
            _stage_a(nc, (sbA, psA), views1, w1_sb, T1, IN)
            nc.sync.dma_start(out=T1[SENT_ROW:SENT_ROW + 1, :], in_=sent1_sb[:])

            # ---- layer 1 edges (D-phase overlaps stage-A1's table build) ----
            pre1 = d_phase(Ts1)
            edge_layer(T1, pre1, b1r_sb, True)

            # ---- second-half allgather; layer-2 D-phase fills the window ----
            nc.gpsimd.collective_compute(
                "AllGather", mybir.AluOpType.bypass,
                replica_groups=[list(range(NCORES))],
                ins=[o1Tb[:].opt()], outs=[o1Tgb[:].opt()])
            pre2 = d_phase(Ts2)

            # ---- stage A, layer 2 second half ----
            views2 = []
            for r in range(NCORES):
                for p0 in range(SPLIT, PERC, SLA):
                    views2.append(
                        (o1Tgb[r * F:(r + 1) * F,
                               p0 - SPLIT:min(p0 + SLA, PERC) - SPLIT],
                         r * SLAB + p0))
            _stage_a(nc, (sbA, psA), views2, w2_sb, T2, F)
            nc.sync.dma_start(out=T2[SENT_ROW:SENT_ROW + 1, :], in_=sent2_sb[:])

            # ---- layer 2 edges ----
            edge_layer(T2, pre2, b2r_sb, False)

    nc.compile()
    return nc


_CACHE = {}


def _prepare(x, edge_index, W1, att_src1, att_dst1, b1, W2, att_src2,
             att_dst2, b2):
    x = np.asarray(x, np.float32)
    edge_index = np.asarray(edge_index, np.int64)
    key = hash(edge_index.tobytes())
    if key in _CACHE:
        meta, nc = _CACHE[key]
    else:
        meta = _preprocess(edge_index)
        nc = _build_program(meta)
        _CACHE[key] = (meta, nc)

    bf = ml_dtypes.bfloat16

    def waug(W, a_s, a_d):
        # cols 0:128 = W (h-major native); 128:132 = W@a_src; 132:136 = W@a_dst
        W = np.asarray(W, np.float32)
        ws = np.einsum("ihf,hf->ih", W.reshape(-1, H, F),
                       np.asarray(a_s, np.float32))
        wd = np.einsum("ihf,hf->ih", W.reshape(-1, H, F),
                       np.asarray(a_d, np.float32))
        return np.concatenate([W, ws, wd], axis=1).astype(bf)

    # sentinel row: asrc = -350 so exp(LRelu(asrc + adst)) ~ 0; h = 0
    sent = np.zeros((1, RW), bf)
    sent[0, 128:132] = -350.0

    # x columns in g-order (junk cols zero)
    xb = x.astype(bf)
    arr = np.zeros((TROWS, IN), bf)
    arr[meta["row_of"]] = xb
    xTg = np.ascontiguousarray(arr.T)

    common = dict(
        xTg=xTg, w1=waug(W1, att_src1, att_dst1), w2=waug(W2, att_src2, att_dst2),
        identb=np.eye(128, dtype=bf), identf=np.eye(128, dtype=np.float32),
        sent1=sent, sent2=sent,
        b1r=np.broadcast_to(np.asarray(b1, np.float32), (128, F)).copy(),
        b2r=np.broadcast_to(np.asarray(b2, np.float32), (128, F)).copy(),
    )
    in_maps = []
    for c in range(NCORES):
        xTs = np.ascontiguousarray(xb[meta["perms"][c]].T)
        in_maps.append(dict(common, gidx=meta["gidx"][c], xTs=xTs))
    return nc, in_maps, meta


def _assemble(meta, results):
    out = np.empty((N, F), np.float32)
    for c in range(NCORES):
        out[meta["perms"][c]] = results[c]["out2p"]
    return out


def kernel(**inputs):
    nc, in_maps, meta = _prepare(**inputs)
    res = run_bass_kernel_spmd(nc, in_maps, core_ids=list(range(NCORES)))
    return _assemble(meta, res.results)


def run_traced(**inputs):
    """Profiled run; returns BassKernelResults (exec_time_ns etc.)."""
    nc, in_maps, meta = _prepare(**inputs)
    res = run_bass_kernel_spmd(nc, in_maps, core_ids=list(range(NCORES)),
                               trace=True)
    res.gat_output = _assemble(meta, res.results)
    return res


# revision 43
# speedup vs baseline: 1.1626x; 1.1626x over previous
"""2-layer GAT (PyG GATConv, concat=False, self-loops) on 8 Trainium2 cores.

Design: nodes/edges partitioned by destination across 8 cores; each core
owns a 6250-dst range, dsts sorted by (degree//4, must-lo count) into 49
blocks of 128 PSUM lanes.  Per-edge source data is fetched with dma_gather
from a 512B-per-node table in HBM; each all-bf16 row is the raw psum of
one augmented matmul x @ [W | W@a_src | W@a_dst]: [h x128 | asrc x4 |
adst x4 | pad].  The gather is SWDGE descriptor-emission bound (~8ns/row
on the Q7), so row bytes are nearly free while row COUNT matters: padding
is minimized by a per-lane-balanced split of edges between two overlapping
int16-indexable views (lo=[0,32767), hi=[17241,50008) of the 50008-row
g-space table - 8 slabs of 6251 rows, one junk/sentinel row per slab), by
excluding self-loop edges (each core streams its own dst rows sequentially,
which also provides a_dst and the self contribution), and by one shared
index stream for both layers.  Padding slots point at the sentinel row
(asrc=-350 => exp(LRelu(...)) ~ 0, h=0).  Scatter is free: slot columns
accumulate into per-block PSUM via identity-weight matmuls, 3 columns per
matmul.  Gather calls batch 15 columns (two calls fit the enlarged 64KB
SWDGE descriptor ring, overlapping emission with drain; multi-packet mode
lifts the 1008-index single-packet limit).  Layer boundary: the layer-1
epilogue emits a transposed bf16 output in two column halves; each half is
AllGathered separately (the first mid-layer) and a replicated stage-A
rebuilds the layer-2 table, overlapping layer-1's tail.  Per-block dst
reads (D-phase) are precomputed during idle stage-A windows.
"""
import sys
sys.path.insert(0, "/opt/trn_rl_repo")

import numpy as np
import ml_dtypes

import concourse.bass as bass
import concourse.bacc as bacc
import concourse.mybir as mybir
from concourse.bass_utils import run_bass_kernel_spmd
from concourse.tile import TileContext

N = 50000
E = 1600000
IN = 128
H = 4
F = 32
NEG = 0.2
NCORES = 8
PERC = N // NCORES           # 6250
SLAB = PERC + 1              # 6251 (junk row at pos 6250)
TROWS = NCORES * SLAB        # 50008
NBLK = (PERC + 127) // 128   # 49
LO_END = 32767               # lo view = rows [0, 32767)
HI_START = TROWS - 32767     # 17241; hi view = rows [17241, 50008)
SENT_ROW = 3 * SLAB + PERC   # 25003 (core 3's junk row; inside the overlap)
RW = 256                     # table row: 256 bf16 slots = 512B:
                             # [h x128 | asrc x4 | adst x4 | 120 pad]
MAXC = int(__import__('os').environ.get('GAT_MAXC', '15'))  # max gather columns per dma_gather call
GRP = 3                      # aggregation matmul: 3 slot-columns per matmul


def _chunks(n):
    return [min(MAXC, n - s) for s in range(0, n, MAXC)]


def _pack_idx(idx_flat):
    """[n] -> [128, n/16] int16; idx i -> (partition i%16, col i//16), x8."""
    n = idx_flat.shape[0]
    assert n % 16 == 0
    a = idx_flat.reshape(n // 16, 16).T.astype(np.int16)
    return np.ascontiguousarray(np.tile(a, (8, 1)))


def _preprocess(edge_index):
    """Index preprocessing. Returns per-core gidx + shared structure."""
    src0 = edge_index[0].astype(np.int64)
    dst0 = edge_index[1].astype(np.int64)
    deg = np.bincount(dst0, minlength=N)          # gather-degree (no self)

    def build_perms(lo_cnt):
        perms = []
        pos_of = np.empty(N, np.int64)
        for c in range(NCORES):
            ids = np.arange(c * PERC, (c + 1) * PERC)
            order = np.lexsort((-lo_cnt[ids], -(deg[ids] // 4)))
            perm = ids[order]
            perms.append(perm)
            pos_of[perm] = np.arange(PERC)
        return perms, pos_of

    # pass 1: degree only -> provisional rows -> per-dst must-lo counts
    perms, pos_of = build_perms(np.zeros(N, np.int64))
    for _ in range(2):
        row_of = (np.arange(N) // PERC) * SLAB + pos_of
        must_lo = row_of[src0] < HI_START
        lo_cnt = np.bincount(dst0[must_lo], minlength=N)
        perms, pos_of = build_perms(lo_cnt)

    row_of = (np.arange(N) // PERC) * SLAB + pos_of
    r_src = row_of[src0]
    kind = np.where(r_src < HI_START, 0, np.where(r_src >= LO_END, 1, 2))

    lane_all = pos_of[dst0]
    core_all = dst0 // PERC
    blk_all = lane_all // 128

    # per-block slot counts nl/nh (cross-core maxima)
    A = np.zeros(NBLK, np.int64)
    B = np.zeros(NBLK, np.int64)
    C = np.zeros(NBLK, np.int64)
    cnt_l = np.zeros((NCORES, PERC), np.int64)
    cnt_h = np.zeros((NCORES, PERC), np.int64)
    cnt_f = np.zeros((NCORES, PERC), np.int64)
    for c in range(NCORES):
        m = core_all == c
        lane = lane_all[m]
        k = kind[m]
        cnt_l[c] = np.bincount(lane[k == 0], minlength=PERC)
        cnt_h[c] = np.bincount(lane[k == 1], minlength=PERC)
        cnt_f[c] = np.bincount(lane[k == 2], minlength=PERC)
    for b in range(NBLK):
        sl = slice(b * 128, min((b + 1) * 128, PERC))
        A[b] = cnt_l[:, sl].max()
        B[b] = cnt_h[:, sl].max()
        C[b] = (cnt_l[:, sl] + cnt_h[:, sl] + cnt_f[:, sl]).max()
    nl = A.copy()
    nh = np.maximum(B, C - A)
    S = int((nl + nh).sum())
    col_lo = np.zeros(NBLK, np.int64)   # column start of block's lo run
    col_hi = np.zeros(NBLK, np.int64)
    col = 0
    for b in range(NBLK):
        col_lo[b] = col
        col += nl[b]
        col_hi[b] = col
        col += nh[b]
    assert col == S

    # per-core column fill
    gidx = []
    sent_lo, sent_hi = SENT_ROW, SENT_ROW - HI_START
    for c in range(NCORES):
        m = core_all == c
        lane = lane_all[m]
        k = kind[m].copy()
        rows = r_src[m]
        # flex -> lo for the first a_i of each lane, rest -> hi
        nh_of_lane = nh[np.arange(PERC) // 128]
        a_i = np.maximum(0, cnt_h[c] + cnt_f[c] - nh_of_lane)
        # order edges by (lane, kind-rank, row); flex edges get rank by
        # position so the first a_i go to lo
        order = np.lexsort((rows, k, lane))
        lane_s, k_s, rows_s = lane[order], k[order], rows[order]
        # cumcount within (lane, kind)
        start = np.r_[True, (lane_s[1:] != lane_s[:-1]) | (k_s[1:] != k_s[:-1])]
        grp_id = np.cumsum(start) - 1
        first_pos = np.full(grp_id[-1] + 1, 1 << 62, np.int64)
        np.minimum.at(first_pos, grp_id, np.arange(len(grp_id)))
        cum = np.arange(len(grp_id)) - first_pos[grp_id]
        # stream + slot per edge
        is_lo = (k_s == 0) | ((k_s == 2) & (cum < a_i[lane_s]))
        slot = np.where(
            k_s == 0, cum,                                   # must-lo
            np.where(k_s == 1, cum,                          # must-hi
                     np.where(is_lo, cnt_l[c][lane_s] + cum,  # flex->lo
                              cnt_h[c][lane_s] + cum - a_i[lane_s])))  # flex->hi
        b_s = lane_s // 128
        colidx = np.where(is_lo, col_lo[b_s] + slot, col_hi[b_s] + slot)
        grid = np.full((S, 128), -1, np.int64)
        grid[colidx, lane_s % 128] = rows_s
        # sentinels + view-relative index
        lo_cols = np.zeros(S, bool)
        for b in range(NBLK):
            lo_cols[col_lo[b]:col_lo[b] + nl[b]] = True
        grid[lo_cols] = np.where(grid[lo_cols] < 0, sent_lo, grid[lo_cols])
        grid[~lo_cols] = np.where(grid[~lo_cols] < 0, sent_hi + HI_START,
                                  grid[~lo_cols]) - HI_START
        assert grid.min() >= 0 and grid.max() < LO_END
        gidx.append(_pack_idx(grid.reshape(-1)))

    return dict(gidx=gidx, nl=nl, nh=nh, S=S, col_lo=col_lo, col_hi=col_hi,
                perms=perms, pos_of=pos_of, row_of=row_of)


SLA = 8 * 128   # stage-A slab width


def _stage_a(nc, pools, views, w_sb, tbl, kdim):
    """[h | asrc | adst] = lhsT.T @ Waug -> rows packed into tbl.

    views: list of (lhsT_view [kdim, W], row0); W <= SLA.  Row layout
    (all bf16, matching the augmented-matmul psum column order):
    [h x128 | asrc x4 | adst x4 | 120 slots garbage].
    """
    sb, ps = pools
    f32, bf16 = mybir.dt.float32, mybir.dt.bfloat16
    for (view, row0) in views:
        Wt = view.shape[1]
        nt = (Wt + 127) // 128
        xsb = sb.tile([kdim, SLA], bf16, tag="xa")
        nc.sync.dma_start(out=xsb[:, 0:Wt], in_=view)
        stg = sb.tile([128, SLA // 128, RW], bf16, tag="sa")
        for t in range(nt):
            w = min(128, Wt - t * 128)
            psum = ps.tile([128, 136], f32, tag="pa")
            nc.tensor.matmul(out=psum[0:w, :], lhsT=xsb[:, t * 128:t * 128 + w],
                             rhs=w_sb[:], start=True, stop=True)
            nc.vector.tensor_copy(out=stg[0:w, t, 0:136], in_=psum[0:w, 0:136])
        nfull = Wt // 128
        if nfull:
            nc.sync.dma_start(
                out=tbl[row0:row0 + nfull * 128, :].rearrange(
                    "(t p) c -> p t c", p=128),
                in_=stg[:, 0:nfull, :])
        if Wt % 128:
            nc.sync.dma_start(out=tbl[row0 + nfull * 128:row0 + Wt, :],
                              in_=stg[0:Wt % 128, nfull, :])


def _build_program(meta):
    nl, nh, S = meta["nl"], meta["nh"], meta["S"]
    import os as _os
    NSWQ = int(_os.environ.get('GAT_NSWQ', '1'))
    nc = bacc.Bacc("TRN2", target_bir_lowering=False, debug=False,
                   num_devices=NCORES, num_swdge_queues=NSWQ,
                   dynamic_dma_scratch_size=int(_os.environ.get('GAT_SCRATCH', '65536')))

    f32, bf16, i16 = mybir.dt.float32, mybir.dt.bfloat16, mybir.dt.int16
    xTg = nc.dram_tensor("xTg", [IN, TROWS], bf16, kind="ExternalInput")
    xTs = nc.dram_tensor("xTs", [IN, PERC], bf16, kind="ExternalInput")
    w1 = nc.dram_tensor("w1", [IN, 136], bf16, kind="ExternalInput")
    w2 = nc.dram_tensor("w2", [F, 136], bf16, kind="ExternalInput")
    gidx = nc.dram_tensor("gidx", [128, S * 8], i16, kind="ExternalInput")
    identb = nc.dram_tensor("identb", [128, 128], bf16, kind="ExternalInput")
    identf = nc.dram_tensor("identf", [128, 128], f32, kind="ExternalInput")
    sent1 = nc.dram_tensor("sent1", [1, RW], bf16, kind="ExternalInput")
    sent2 = nc.dram_tensor("sent2", [1, RW], bf16, kind="ExternalInput")
    b1r = nc.dram_tensor("b1r", [128, F], f32, kind="ExternalInput")
    b2r = nc.dram_tensor("b2r", [128, F], f32, kind="ExternalInput")

    T1 = nc.dram_tensor("T1", [TROWS, RW], bf16)
    T2 = nc.dram_tensor("T2", [TROWS, RW], bf16)
    Ts1 = nc.dram_tensor("Ts1", [PERC, RW], bf16)
    Ts2 = nc.dram_tensor("Ts2", [PERC, RW], bf16)
    SPLIT = 24 * 128             # o1T column split (block 24 boundary)
    o1Ta = nc.dram_tensor("o1Ta", [F, SPLIT], bf16)
    o1Tb = nc.dram_tensor("o1Tb", [F, PERC - SPLIT], bf16)
    o1Tga = nc.dram_tensor("o1Tga", [NCORES * F, SPLIT], bf16,
                           addr_space="Shared")
    o1Tgb = nc.dram_tensor("o1Tgb", [NCORES * F, PERC - SPLIT], bf16,
                           addr_space="Shared")
    out2p = nc.dram_tensor("out2p", [PERC, F], f32, kind="ExternalOutput")

    NC3 = ((MAXC + GRP - 1) // GRP) * GRP  # rhs tile columns (33)

    with TileContext(nc) as tc:
        with (
            tc.tile_pool(name="cons", bufs=1) as cons,
            tc.tile_pool(name="sbA", bufs=3) as sbA,
            tc.tile_pool(name="psA", bufs=2, space="PSUM") as psA,
            tc.tile_pool(name="dp", bufs=2) as dp,
            tc.tile_pool(name="gp", bufs=4) as gp,
            tc.tile_pool(name="rp", bufs=3) as rp,
            tc.tile_pool(name="ep", bufs=8) as ep,
            tc.tile_pool(name="pp", bufs=1) as pp,
            tc.tile_pool(name="psE", bufs=3, space="PSUM") as psE,
            tc.tile_pool(name="psT", bufs=1, space="PSUM") as psT,
        ):
            identb_sb = cons.tile([128, 128], bf16)
            nc.sync.dma_start(out=identb_sb[:], in_=identb[:, :])
            identf_sb = cons.tile([128, 128], f32)
            nc.sync.dma_start(out=identf_sb[:], in_=identf[:, :])
            w1_sb = cons.tile([IN, 136], bf16)
            nc.sync.dma_start(out=w1_sb[:], in_=w1[:, :])
            w2_sb = cons.tile([F, 136], bf16)
            nc.sync.dma_start(out=w2_sb[:], in_=w2[:, :])
            b1r_sb = cons.tile([128, F], f32)
            nc.sync.dma_start(out=b1r_sb[:], in_=b1r[:, :])
            b2r_sb = cons.tile([128, F], f32)
            nc.sync.dma_start(out=b2r_sb[:], in_=b2r[:, :])
            sent1_sb = cons.tile([1, RW], bf16)
            nc.sync.dma_start(out=sent1_sb[:], in_=sent1[:, :])
            sent2_sb = cons.tile([1, RW], bf16)
            nc.sync.dma_start(out=sent2_sb[:], in_=sent2[:, :])
            gidx_sb = cons.tile([128, S * 8], i16)
            nc.sync.dma_start(out=gidx_sb[:], in_=gidx[:, :])

            def d_phase(tself):
                """Per-block dst-slab read; returns pre[b] = (adl, rhs_s)
                tiles that persist until the g-phase."""
                pre = {}
                for b in range(NBLK):
                    w_b = min(128, PERC - b * 128)
                    dt = dp.tile([128, RW], bf16, tag="dt")
                    if w_b < 128:
                        nc.vector.memset(dt[:], 0.0)
                    nc.sync.dma_start(out=dt[0:w_b, :],
                                      in_=tself[b * 128:b * 128 + w_b, :])
                    dhb = dt[:, 0:128]                 # [128, 128] h-major
                    adl = pp.tile([128, H], bf16, tag=f"adl{b}")
                    nc.vector.tensor_copy(out=adl[:], in_=dt[:, 132:136])
                    # self edge: e = LRelu(asrc + adst); w = exp(e)
                    es = ep.tile([128, H], f32, tag="es")
                    nc.vector.tensor_tensor(out=es[:], in0=dt[:, 128:132],
                                            in1=dt[:, 132:136],
                                            op=mybir.AluOpType.add)
                    es2 = ep.tile([128, H], f32, tag="es2")
                    nc.vector.tensor_scalar(out=es2[:], in0=es[:], scalar1=NEG,
                                            scalar2=None, op0=mybir.AluOpType.mult)
                    nc.vector.tensor_tensor(out=es2[:], in0=es2[:], in1=es[:],
                                            op=mybir.AluOpType.max)
                    wsb = ep.tile([128, H], bf16, tag="wsb")
                    nc.scalar.activation(out=wsb[:], in_=es2[:],
                                         func=mybir.ActivationFunctionType.Exp)
                    rhs_s = pp.tile([128, 4 + H * F], f32, tag=f"rs{b}")
                    nc.vector.tensor_copy(out=rhs_s[:, 0:4], in_=wsb[:])
                    nc.vector.tensor_tensor(
                        out=rhs_s[:, 4:132].rearrange("p (h f) -> p h f", f=F),
                        in0=dhb.rearrange("p (h f) -> p h f", f=F),
                        in1=wsb[:].unsqueeze(2).to_broadcast([128, H, F]),
                        op=mybir.AluOpType.mult)
                    pre[b] = (adl, rhs_s)
                return pre

            qctr = [0]

            def edge_layer(tbl, pre, bias_sb, is_layer1):
                tbl_lo = tbl[0:LO_END, :]
                tbl_hi = tbl[HI_START:TROWS, :]
                for b in range(NBLK):
                    w_b = min(128, PERC - b * 128)
                    adl, rhs_s = pre[b]

                    # ---- gathered slots ----
                    psum = psE.tile([128, GRP * 132], f32, tag="acc")
                    n_tri = sum((cc + GRP - 1) // GRP
                                for nn in (int(nl[b]), int(nh[b]))
                                for cc in _chunks(nn))
                    tri = 0
                    for half in range(2):
                        ncols_all = int(nl[b]) if half == 0 else int(nh[b])
                        col0 = int(meta["col_lo"][b]) if half == 0 \
                            else int(meta["col_hi"][b])
                        view = tbl_lo if half == 0 else tbl_hi
                        for s0 in range(0, ncols_all, MAXC):
                            ncc = min(MAXC, ncols_all - s0)
                            nc3 = ((ncc + GRP - 1) // GRP) * GRP
                            g = gp.tile([128, MAXC, RW], bf16, tag="g")
                            nc.gpsimd.dma_gather(
                                g[:, 0:ncc, :], view,
                                gidx_sb[:, (col0 + s0) * 8:(col0 + s0 + ncc) * 8],
                                ncc * 128, ncc * 128, RW,
                                single_packet=(ncc * 128 <= 1008),
                                queue_num=qctr[0] % NSWQ)
                            qctr[0] += 1
                            gh = g[:, 0:ncc, 0:128]               # [*,nc,128]
                            al = ep.tile([128, MAXC, H], f32, tag="al")
                            nc.vector.tensor_tensor(
                                out=al[:, 0:ncc, :], in0=g[:, 0:ncc, 128:132],
                                in1=adl[:].unsqueeze(1).to_broadcast(
                                    [128, ncc, H]),
                                op=mybir.AluOpType.add)
                            alf = al[:, 0:ncc, :].rearrange("p n h -> p (n h)")
                            e2 = ep.tile([128, MAXC, H], f32, tag="e2")
                            e2f = e2[:, 0:ncc, :].rearrange("p n h -> p (n h)")
                            nc.vector.tensor_scalar(
                                out=e2f, in0=alf, scalar1=NEG, scalar2=None,
                                op0=mybir.AluOpType.mult)
                            nc.vector.tensor_tensor(
                                out=e2f, in0=e2f, in1=alf,
                                op=mybir.AluOpType.max)
                            rhs = rp.tile([128, NC3, 132], bf16, tag="rhs")
                            if nc3 > ncc:
                                nc.vector.memset(rhs[:, ncc:nc3, :], 0.0)
                            nc.scalar.activation(
                                out=rhs[:, 0:ncc, 0:4], in_=e2[:, 0:ncc, :],
                                func=mybir.ActivationFunctionType.Exp)
                            nc.vector.tensor_tensor(
                                out=rhs[:, 0:ncc, 4:132].rearrange(
                                    "p n (h f) -> p n h f", f=F),
                                in0=gh.rearrange(
                                    "p n (h f) -> p n h f", f=F),
                                in1=rhs[:, 0:ncc, 0:4].unsqueeze(3).to_broadcast(
                                    [128, ncc, H, F]),
                                op=mybir.AluOpType.mult)
                            for t in range(nc3 // GRP):
                                nc.tensor.matmul(
                                    out=psum[:],
                                    lhsT=identb_sb[:],
                                    rhs=rhs[:, t * GRP:(t + 1) * GRP, :].rearrange(
                                        "p a b -> p (a b)"),
                                    start=(tri == 0), stop=(tri == n_tri - 1))
                                tri += 1
                    assert tri == n_tri

                    # ---- epilogue ----
                    U = ep.tile([128, 132], f32, tag="U")
                    nc.vector.tensor_tensor(out=U[:], in0=rhs_s[:],
                                            in1=psum[:, 0:132],
                                            op=mybir.AluOpType.add)
                    nc.vector.tensor_tensor(out=U[:], in0=U[:],
                                            in1=psum[:, 132:264],
                                            op=mybir.AluOpType.add)
                    nc.vector.tensor_tensor(out=U[:], in0=U[:],
                                            in1=psum[:, 264:396],
                                            op=mybir.AluOpType.add)
                    sden = ep.tile([128, H], f32, tag="sden")
                    nc.vector.tensor_scalar(out=sden[:], in0=U[:, 0:4],
                                            scalar1=1e-16, scalar2=None,
                                            op0=mybir.AluOpType.add)
                    rv = ep.tile([128, H], f32, tag="rv")
                    nc.vector.reciprocal(out=rv[:], in_=sden[:])
                    nc.vector.tensor_scalar(out=rv[:], in0=rv[:], scalar1=1.0 / H,
                                            scalar2=None,
                                            op0=mybir.AluOpType.mult)
                    m = ep.tile([128, H * F], f32, tag="m")
                    nc.vector.tensor_tensor(
                        out=m[:].rearrange("p (h f) -> p h f", f=F),
                        in0=U[:, 4:132].rearrange("p (h f) -> p h f", f=F),
                        in1=rv[:].unsqueeze(2).to_broadcast([128, H, F]),
                        op=mybir.AluOpType.mult)
                    o = ep.tile([128, F], f32, tag="o")
                    nc.vector.tensor_tensor(out=o[:], in0=m[:, 0:F],
                                            in1=m[:, F:2 * F],
                                            op=mybir.AluOpType.add)
                    o2 = ep.tile([128, F], f32, tag="o2t")
                    nc.vector.tensor_tensor(out=o2[:], in0=m[:, 2 * F:3 * F],
                                            in1=m[:, 3 * F:4 * F],
                                            op=mybir.AluOpType.add)
                    nc.vector.tensor_tensor(out=o[:], in0=o[:], in1=o2[:],
                                            op=mybir.AluOpType.add)
                    nc.vector.tensor_tensor(out=o[:], in0=o[:], in1=bias_sb[:],
                                            op=mybir.AluOpType.add)
                    if is_layer1:
                        # ELU
                        m0 = ep.tile([128, F], f32, tag="m0")
                        nc.vector.tensor_scalar(out=m0[:], in0=o[:], scalar1=0.0,
                                                scalar2=None,
                                                op0=mybir.AluOpType.min)
                        em = ep.tile([128, F], f32, tag="em")
                        nc.scalar.activation(out=em[:], in_=m0[:],
                                             func=mybir.ActivationFunctionType.Exp)
                        nc.vector.tensor_scalar(out=em[:], in0=em[:], scalar1=-1.0,
                                                scalar2=None,
                                                op0=mybir.AluOpType.add)
                        nc.vector.tensor_tensor(out=o[:], in0=o[:], in1=em[:],
                                                op=mybir.AluOpType.max)
                        # transpose -> o1T (bf16) + h2 = o1 @ W2 -> Ts2
                        pT = psT.tile([F, 128], f32, tag="pT")
                        nc.tensor.transpose(out=pT[:], in_=o[:],
                                            identity=identf_sb[:])
                        oT = ep.tile([F, 128], bf16, tag="oT")
                        nc.vector.tensor_copy(out=oT[:], in_=pT[:])
                        if b * 128 < SPLIT:
                            nc.sync.dma_start(
                                out=o1Ta[:, b * 128:b * 128 + w_b],
                                in_=oT[:, 0:w_b])
                        else:
                            nc.sync.dma_start(
                                out=o1Tb[:, b * 128 - SPLIT:
                                         b * 128 - SPLIT + w_b],
                                in_=oT[:, 0:w_b])
                        if b == SPLIT // 128 - 1:
                            # first-half allgather + stage-A2 overlap the
                            # remaining layer-1 blocks
                            nc.gpsimd.collective_compute(
                                "AllGather", mybir.AluOpType.bypass,
                                replica_groups=[list(range(NCORES))],
                                ins=[o1Ta[:].opt()], outs=[o1Tga[:].opt()])
                            viewsA = []
                            for r in range(NCORES):
                                for p0 in range(0, SPLIT, SLA):
                                    viewsA.append(
                                        (o1Tga[r * F:(r + 1) * F,
                                               p0:min(p0 + SLA, SPLIT)],
                                         r * SLAB + p0))
                            _stage_a(nc, (sbA, psA), viewsA, w2_sb, T2, F)
                        ps2 = psT.tile([128, 136], f32, tag="ps2")
                        nc.tensor.matmul(out=ps2[:], lhsT=oT[:], rhs=w2_sb[:],
                                         start=True, stop=True)
                        st2 = ep.tile([128, RW], bf16, tag="st2")
                        nc.vector.tensor_copy(out=st2[:, 0:136],
                                              in_=ps2[:, 0:136])
                        nc.sync.dma_start(out=Ts2[b * 128:b * 128 + w_b, :],
                                          in_=st2[0:w_b, :])
                    else:
                        nc.sync.dma_start(out=out2p[b * 128:b * 128 + w_b, :],
                                          in_=o[0:w_b, :])

            # ---- stage A, layer 1: self slab first, then full table ----
            viewsS = [(xTs[:, s0:min(s0 + SLA, PERC)], s0)
                      for s0 in range(0, PERC, SLA)]
            _stage_a(nc, (sbA, psA), viewsS, w1_sb, Ts1, IN)
            views1 = [(xTg[:, s0:min(s0 + SLA, TROWS)], s0)
                      for s0 in range(0, TROWS, SLA)]
            _stage_a(nc, (sbA, psA), views1, w1_sb, T1, IN)
            nc.sync.dma_start(out=T1[SENT_ROW:SENT_ROW + 1, :], in_=sent1_sb[:])

            # ---- layer 1 edges (D-phase overlaps stage-A1's table build) ----
            pre1 = d_phase(Ts1)
            edge_layer(T1, pre1, b1r_sb, True)

            # ---- second-half allgather; layer-2 D-phase fills the window ----
            nc.gpsimd.collective_compute(
                "AllGather", mybir.AluOpType.bypass,
                replica_groups=[list(range(NCORES))],
                ins=[o1Tb[:].opt()], outs=[o1Tgb[:].opt()])
            pre2 = d_phase(Ts2)

            # ---- stage A, layer 2 second half ----
            views2 = []
            for r in range(NCORES):
                for p0 in range(SPLIT, PERC, SLA):
                    views2.append(
                        (o1Tgb[r * F:(r + 1) * F,
                               p0 - SPLIT:min(p0 + SLA, PERC) - SPLIT],
                         r * SLAB + p0))
            _stage_a(nc, (sbA, psA), views2, w2_sb, T2, F)
            nc.sync.dma_start(out=T2[SENT_ROW:SENT_ROW + 1, :], in_=sent2_sb[:])

            # ---- layer 2 edges ----
            edge_layer(T2, pre2, b2r_sb, False)

    nc.compile()
    return nc


_CACHE = {}


def _prepare(x, edge_index, W1, att_src1, att_dst1, b1, W2, att_src2,
             att_dst2, b2):
    x = np.asarray(x, np.float32)
    edge_index = np.asarray(edge_index, np.int64)
    key = hash(edge_index.tobytes())
    if key in _CACHE:
        meta, nc = _CACHE[key]
    else:
        meta = _preprocess(edge_index)
        nc = _build_program(meta)
        _CACHE[key] = (meta, nc)

    bf = ml_dtypes.bfloat16

    def waug(W, a_s, a_d):
        # cols 0:128 = W (h-major native); 128:132 = W@a_src; 132:136 = W@a_dst
        W = np.asarray(W, np.float32)
        ws = np.einsum("ihf,hf->ih", W.reshape(-1, H, F),
                       np.asarray(a_s, np.float32))
        wd = np.einsum("ihf,hf->ih", W.reshape(-1, H, F),
                       np.asarray(a_d, np.float32))
        return np.concatenate([W, ws, wd], axis=1).astype(bf)

    # sentinel row: asrc = -350 so exp(LRelu(asrc + adst)) ~ 0; h = 0
    sent = np.zeros((1, RW), bf)
    sent[0, 128:132] = -350.0

    # x columns in g-order (junk cols zero)
    xb = x.astype(bf)
    arr = np.zeros((TROWS, IN), bf)
    arr[meta["row_of"]] = xb
    xTg = np.ascontiguousarray(arr.T)

    common = dict(
        xTg=xTg, w1=waug(W1, att_src1, att_dst1), w2=waug(W2, att_src2, att_dst2),
        identb=np.eye(128, dtype=bf), identf=np.eye(128, dtype=np.float32),
        sent1=sent, sent2=sent,
        b1r=np.broadcast_to(np.asarray(b1, np.float32), (128, F)).copy(),
        b2r=np.broadcast_to(np.asarray(b2, np.float32), (128, F)).copy(),
    )
    in_maps = []
    for c in range(NCORES):
        xTs = np.ascontiguousarray(xb[meta["perms"][c]].T)
        in_maps.append(dict(common, gidx=meta["gidx"][c], xTs=xTs))
    return nc, in_maps, meta


def _assemble(meta, results):
    out = np.empty((N, F), np.float32)
    for c in range(NCORES):
        out[meta["perms"][c]] = results[c]["out2p"]
    return out


def kernel(**inputs):
    nc, in_maps, meta = _prepare(**inputs)
    res = run_bass_kernel_spmd(nc, in_maps, core_ids=list(range(NCORES)))
    return _assemble(meta, res.results)


def run_traced(**inputs):
    """Profiled run; returns BassKernelResults (exec_time_ns etc.)."""
    nc, in_maps, meta = _prepare(**inputs)
    res = run_bass_kernel_spmd(nc, in_maps, core_ids=list(range(NCORES)),
                               trace=True)
    res.gat_output = _assemble(meta, res.results)
    return res


# revision 46
# speedup vs baseline: 1.2101x; 1.0409x over previous
"""2-layer GAT (PyG GATConv, concat=False, self-loops) on 8 Trainium2 cores.

Design: nodes/edges partitioned by destination across 8 cores; each core
owns a 6250-dst range, dsts sorted by (degree//4, must-lo count) into 49
blocks of 128 PSUM lanes.  Per-edge source data is fetched with dma_gather
from a 512B-per-node table in HBM; each all-bf16 row is the raw psum of
one augmented matmul x @ [W | W@a_src | W@a_dst]: [h x128 | asrc x4 |
adst x4 | pad].  The gather is SWDGE descriptor-emission bound (~8ns/row
on the Q7), so row bytes are nearly free while row COUNT matters: padding
is minimized by a per-lane-balanced split of edges between two overlapping
int16-indexable views (lo=[0,32767), hi=[17241,50008) of the 50008-row
g-space table - 8 slabs of 6251 rows, one junk/sentinel row per slab), by
excluding self-loop edges (each core streams its own dst rows sequentially,
which also provides a_dst and the self contribution), and by one shared
index stream for both layers.  Padding slots point at the sentinel row
(asrc=-350 => exp(LRelu(...)) ~ 0, h=0).  Scatter is free: slot columns
accumulate into per-block PSUM via identity-weight matmuls, 3 columns per
matmul.  Gather calls batch 15 columns (two calls fit the enlarged 64KB
SWDGE descriptor ring, overlapping emission with drain; multi-packet mode
lifts the 1008-index single-packet limit).  Layer boundary: the layer-1
epilogue emits a transposed bf16 output in two column halves; each half is
AllGathered separately (the first mid-layer) and a replicated stage-A
rebuilds the layer-2 table, overlapping layer-1's tail.  Per-block dst
reads (D-phase) are precomputed during idle stage-A windows.
"""
import sys
sys.path.insert(0, "/opt/trn_rl_repo")

import numpy as np
import ml_dtypes

import concourse.bass as bass
import concourse.bacc as bacc
import concourse.mybir as mybir
from concourse.bass_utils import run_bass_kernel_spmd
from concourse.tile import TileContext

N = 50000
E = 1600000
IN = 128
H = 4
F = 32
NEG = 0.2
NCORES = 8
PERC = N // NCORES           # 6250
SLAB = PERC + 1              # 6251 (junk row at pos 6250)
TROWS = NCORES * SLAB        # 50008
NBLK = (PERC + 127) // 128   # 49
LO_END = 32767               # lo view = rows [0, 32767)
HI_START = TROWS - 32767     # 17241; hi view = rows [17241, 50008)
SENT_ROW = 3 * SLAB + PERC   # 25003 (core 3's junk row; inside the overlap)
RW = 256                     # table row: 256 bf16 slots = 512B:
                             # [h x128 | asrc x4 | adst x4 | 120 pad]
MAXC = int(__import__('os').environ.get('GAT_MAXC', '15'))  # max gather columns per dma_gather call
GRP = 3                      # aggregation matmul: 3 slot-columns per matmul


def _chunks(n):
    return [min(MAXC, n - s) for s in range(0, n, MAXC)]


def _pack_idx(idx_flat):
    """[n] -> [128, n/16] int16; idx i -> (partition i%16, col i//16), x8."""
    n = idx_flat.shape[0]
    assert n % 16 == 0
    a = idx_flat.reshape(n // 16, 16).T.astype(np.int16)
    return np.ascontiguousarray(np.tile(a, (8, 1)))


def _preprocess(edge_index):
    """Index preprocessing. Returns per-core gidx + shared structure."""
    src0 = edge_index[0].astype(np.int64)
    dst0 = edge_index[1].astype(np.int64)
    deg = np.bincount(dst0, minlength=N)          # gather-degree (no self)

    def build_perms(lo_cnt):
        perms = []
        pos_of = np.empty(N, np.int64)
        for c in range(NCORES):
            ids = np.arange(c * PERC, (c + 1) * PERC)
            order = np.lexsort((-lo_cnt[ids], -(deg[ids] // 4)))
            perm = ids[order]
            perms.append(perm)
            pos_of[perm] = np.arange(PERC)
        return perms, pos_of

    # pass 1: degree only -> provisional rows -> per-dst must-lo counts
    perms, pos_of = build_perms(np.zeros(N, np.int64))
    for _ in range(2):
        row_of = (np.arange(N) // PERC) * SLAB + pos_of
        must_lo = row_of[src0] < HI_START
        lo_cnt = np.bincount(dst0[must_lo], minlength=N)
        perms, pos_of = build_perms(lo_cnt)

    row_of = (np.arange(N) // PERC) * SLAB + pos_of
    r_src = row_of[src0]
    kind = np.where(r_src < HI_START, 0, np.where(r_src >= LO_END, 1, 2))

    lane_all = pos_of[dst0]
    core_all = dst0 // PERC
    blk_all = lane_all // 128

    # per-block slot counts nl/nh (cross-core maxima)
    A = np.zeros(NBLK, np.int64)
    B = np.zeros(NBLK, np.int64)
    C = np.zeros(NBLK, np.int64)
    cnt_l = np.zeros((NCORES, PERC), np.int64)
    cnt_h = np.zeros((NCORES, PERC), np.int64)
    cnt_f = np.zeros((NCORES, PERC), np.int64)
    for c in range(NCORES):
        m = core_all == c
        lane = lane_all[m]
        k = kind[m]
        cnt_l[c] = np.bincount(lane[k == 0], minlength=PERC)
        cnt_h[c] = np.bincount(lane[k == 1], minlength=PERC)
        cnt_f[c] = np.bincount(lane[k == 2], minlength=PERC)
    for b in range(NBLK):
        sl = slice(b * 128, min((b + 1) * 128, PERC))
        A[b] = cnt_l[:, sl].max()
        B[b] = cnt_h[:, sl].max()
        C[b] = (cnt_l[:, sl] + cnt_h[:, sl] + cnt_f[:, sl]).max()
    nl = A.copy()
    nh = np.maximum(B, C - A)
    S = int((nl + nh).sum())
    col_lo = np.zeros(NBLK, np.int64)   # column start of block's lo run
    col_hi = np.zeros(NBLK, np.int64)
    col = 0
    for b in range(NBLK):
        col_lo[b] = col
        col += nl[b]
        col_hi[b] = col
        col += nh[b]
    assert col == S

    # per-core column fill
    gidx = []
    sent_lo, sent_hi = SENT_ROW, SENT_ROW - HI_START
    for c in range(NCORES):
        m = core_all == c
        lane = lane_all[m]
        k = kind[m].copy()
        rows = r_src[m]
        # flex -> lo for the first a_i of each lane, rest -> hi
        nh_of_lane = nh[np.arange(PERC) // 128]
        a_i = np.maximum(0, cnt_h[c] + cnt_f[c] - nh_of_lane)
        # order edges by (lane, kind-rank, row); flex edges get rank by
        # position so the first a_i go to lo
        order = np.lexsort((rows, k, lane))
        lane_s, k_s, rows_s = lane[order], k[order], rows[order]
        # cumcount within (lane, kind)
        start = np.r_[True, (lane_s[1:] != lane_s[:-1]) | (k_s[1:] != k_s[:-1])]
        grp_id = np.cumsum(start) - 1
        first_pos = np.full(grp_id[-1] + 1, 1 << 62, np.int64)
        np.minimum.at(first_pos, grp_id, np.arange(len(grp_id)))
        cum = np.arange(len(grp_id)) - first_pos[grp_id]
        # stream + slot per edge
        is_lo = (k_s == 0) | ((k_s == 2) & (cum < a_i[lane_s]))
        slot = np.where(
            k_s == 0, cum,                                   # must-lo
            np.where(k_s == 1, cum,                          # must-hi
                     np.where(is_lo, cnt_l[c][lane_s] + cum,  # flex->lo
                              cnt_h[c][lane_s] + cum - a_i[lane_s])))  # flex->hi
        b_s = lane_s // 128
        colidx = np.where(is_lo, col_lo[b_s] + slot, col_hi[b_s] + slot)
        grid = np.full((S, 128), -1, np.int64)
        grid[colidx, lane_s % 128] = rows_s
        # sentinels + view-relative index
        lo_cols = np.zeros(S, bool)
        for b in range(NBLK):
            lo_cols[col_lo[b]:col_lo[b] + nl[b]] = True
        grid[lo_cols] = np.where(grid[lo_cols] < 0, sent_lo, grid[lo_cols])
        grid[~lo_cols] = np.where(grid[~lo_cols] < 0, sent_hi + HI_START,
                                  grid[~lo_cols]) - HI_START
        assert grid.min() >= 0 and grid.max() < LO_END
        gidx.append(_pack_idx(grid.reshape(-1)))

    return dict(gidx=gidx, nl=nl, nh=nh, S=S, col_lo=col_lo, col_hi=col_hi,
                perms=perms, pos_of=pos_of, row_of=row_of)


SLA = 8 * 128   # stage-A slab width


def _stage_a(nc, pools, views, w_sb, tbl, kdim):
    """[h | asrc | adst] = lhsT.T @ Waug -> rows packed into tbl.

    views: list of (lhsT_view [kdim, W], row0); W <= SLA.  Row layout
    (all bf16, matching the augmented-matmul psum column order):
    [h x128 | asrc x4 | adst x4 | 120 slots garbage].
    """
    sb, ps = pools
    f32, bf16 = mybir.dt.float32, mybir.dt.bfloat16
    for (view, row0) in views:
        Wt = view.shape[1]
        nt = (Wt + 127) // 128
        xsb = sb.tile([kdim, SLA], bf16, tag="xa")
        nc.sync.dma_start(out=xsb[:, 0:Wt], in_=view)
        stg = sb.tile([128, SLA // 128, RW], bf16, tag="sa")
        for t in range(nt):
            w = min(128, Wt - t * 128)
            psum = ps.tile([128, 136], f32, tag="pa")
            nc.tensor.matmul(out=psum[0:w, :], lhsT=xsb[:, t * 128:t * 128 + w],
                             rhs=w_sb[:], start=True, stop=True)
            nc.vector.tensor_copy(out=stg[0:w, t, 0:136], in_=psum[0:w, 0:136])
        nfull = Wt // 128
        if nfull:
            nc.sync.dma_start(
                out=tbl[row0:row0 + nfull * 128, :].rearrange(
                    "(t p) c -> p t c", p=128),
                in_=stg[:, 0:nfull, :])
        if Wt % 128:
            nc.sync.dma_start(out=tbl[row0 + nfull * 128:row0 + Wt, :],
                              in_=stg[0:Wt % 128, nfull, :])


def _build_program(meta):
    nl, nh, S = meta["nl"], meta["nh"], meta["S"]
    import os as _os
    NSWQ = int(_os.environ.get('GAT_NSWQ', '1'))
    nc = bacc.Bacc("TRN2", target_bir_lowering=False, debug=False,
                   num_devices=NCORES, num_swdge_queues=NSWQ,
                   dynamic_dma_scratch_size=int(_os.environ.get('GAT_SCRATCH', '65536')))

    f32, bf16, i16 = mybir.dt.float32, mybir.dt.bfloat16, mybir.dt.int16
    xTg = nc.dram_tensor("xTg", [IN, TROWS], bf16, kind="ExternalInput")
    xTs = nc.dram_tensor("xTs", [IN, PERC], bf16, kind="ExternalInput")
    w1 = nc.dram_tensor("w1", [IN, 136], bf16, kind="ExternalInput")
    w2 = nc.dram_tensor("w2", [F, 136], bf16, kind="ExternalInput")
    gidx = nc.dram_tensor("gidx", [128, S * 8], i16, kind="ExternalInput")
    identb = nc.dram_tensor("identb", [128, 128], bf16, kind="ExternalInput")
    identf = nc.dram_tensor("identf", [128, 128], f32, kind="ExternalInput")
    sent1 = nc.dram_tensor("sent1", [1, RW], bf16, kind="ExternalInput")
    sent2 = nc.dram_tensor("sent2", [1, RW], bf16, kind="ExternalInput")
    b1r = nc.dram_tensor("b1r", [128, F], f32, kind="ExternalInput")
    b2r = nc.dram_tensor("b2r", [128, F], f32, kind="ExternalInput")

    T1 = nc.dram_tensor("T1", [TROWS, RW], bf16)
    T2 = nc.dram_tensor("T2", [TROWS, RW], bf16)
    Ts1 = nc.dram_tensor("Ts1", [PERC, RW], bf16)
    Ts2 = nc.dram_tensor("Ts2", [PERC, RW], bf16)
    SPLIT = 24 * 128             # o1T column split (block 24 boundary)
    o1Ta = nc.dram_tensor("o1Ta", [F, SPLIT], bf16)
    o1Tb = nc.dram_tensor("o1Tb", [F, PERC - SPLIT], bf16)
    o1Tga = nc.dram_tensor("o1Tga", [NCORES * F, SPLIT], bf16,
                           addr_space="Shared")
    o1Tgb = nc.dram_tensor("o1Tgb", [NCORES * F, PERC - SPLIT], bf16,
                           addr_space="Shared")
    out2p = nc.dram_tensor("out2p", [PERC, F], f32, kind="ExternalOutput")

    NC3 = ((MAXC + GRP - 1) // GRP) * GRP  # rhs tile columns (33)

    with TileContext(nc) as tc:
        with (
            tc.tile_pool(name="cons", bufs=1) as cons,
            tc.tile_pool(name="sbA", bufs=3) as sbA,
            tc.tile_pool(name="psA", bufs=2, space="PSUM") as psA,
            tc.tile_pool(name="dp", bufs=2) as dp,
            tc.tile_pool(name="gp", bufs=4) as gp,
            tc.tile_pool(name="rp", bufs=3) as rp,
            tc.tile_pool(name="ep", bufs=8) as ep,
            tc.tile_pool(name="pp", bufs=1) as pp,
            tc.tile_pool(name="psE", bufs=3, space="PSUM") as psE,
            tc.tile_pool(name="psT", bufs=1, space="PSUM") as psT,
        ):
            identb_sb = cons.tile([128, 128], bf16)
            nc.sync.dma_start(out=identb_sb[:], in_=identb[:, :])
            identf_sb = cons.tile([128, 128], f32)
            nc.sync.dma_start(out=identf_sb[:], in_=identf[:, :])
            w1_sb = cons.tile([IN, 136], bf16)
            nc.sync.dma_start(out=w1_sb[:], in_=w1[:, :])
            w2_sb = cons.tile([F, 136], bf16)
            nc.sync.dma_start(out=w2_sb[:], in_=w2[:, :])
            b1r_sb = cons.tile([128, F], f32)
            nc.sync.dma_start(out=b1r_sb[:], in_=b1r[:, :])
            b2r_sb = cons.tile([128, F], f32)
            nc.sync.dma_start(out=b2r_sb[:], in_=b2r[:, :])
            sent1_sb = cons.tile([1, RW], bf16)
            nc.sync.dma_start(out=sent1_sb[:], in_=sent1[:, :])
            sent2_sb = cons.tile([1, RW], bf16)
            nc.sync.dma_start(out=sent2_sb[:], in_=sent2[:, :])
            gidx_sb = cons.tile([128, S * 8], i16)
            nc.sync.dma_start(out=gidx_sb[:], in_=gidx[:, :])

            def d_block(tself, b):
                """One block's dst-slab read + self-edge prep; returns
                (adl, rhs_s) tiles that persist until the epilogue."""
                if True:
                    w_b = min(128, PERC - b * 128)
                    dt = dp.tile([128, RW], bf16, tag="dt")
                    if w_b < 128:
                        nc.vector.memset(dt[:], 0.0)
                    nc.sync.dma_start(out=dt[0:w_b, :],
                                      in_=tself[b * 128:b * 128 + w_b, :])
                    dhb = dt[:, 0:128]                 # [128, 128] h-major
                    adl = pp.tile([128, H], bf16, tag=f"adl{b}")
                    nc.vector.tensor_copy(out=adl[:], in_=dt[:, 132:136])
                    # self edge: e = LRelu(asrc + adst); w = exp(e)
                    es = ep.tile([128, H], f32, tag="es")
                    nc.vector.tensor_tensor(out=es[:], in0=dt[:, 128:132],
                                            in1=dt[:, 132:136],
                                            op=mybir.AluOpType.add)
                    es2 = ep.tile([128, H], f32, tag="es2")
                    nc.vector.tensor_scalar(out=es2[:], in0=es[:], scalar1=NEG,
                                            scalar2=None, op0=mybir.AluOpType.mult)
                    nc.vector.tensor_tensor(out=es2[:], in0=es2[:], in1=es[:],
                                            op=mybir.AluOpType.max)
                    wsb = ep.tile([128, H], bf16, tag="wsb")
                    nc.scalar.activation(out=wsb[:], in_=es2[:],
                                         func=mybir.ActivationFunctionType.Exp)
                    rhs_s = pp.tile([128, 4 + H * F], f32, tag=f"rs{b}")
                    nc.vector.tensor_copy(out=rhs_s[:, 0:4], in_=wsb[:])
                    nc.vector.tensor_tensor(
                        out=rhs_s[:, 4:132].rearrange("p (h f) -> p h f", f=F),
                        in0=dhb.rearrange("p (h f) -> p h f", f=F),
                        in1=wsb[:].unsqueeze(2).to_broadcast([128, H, F]),
                        op=mybir.AluOpType.mult)
                    return (adl, rhs_s)

            def d_phase(tself):
                return {b: d_block(tself, b) for b in range(NBLK)}

            qctr = [0]

            def edge_layer(tbl, pre, bias_sb, is_layer1, tself=None):
                tbl_lo = tbl[0:LO_END, :]
                tbl_hi = tbl[HI_START:TROWS, :]
                for b in range(NBLK):
                    w_b = min(128, PERC - b * 128)
                    adl, rhs_s = pre[b] if pre is not None else \
                        d_block(tself, b)

                    # ---- gathered slots ----
                    psum = psE.tile([128, GRP * 132], f32, tag="acc")
                    n_tri = sum((cc + GRP - 1) // GRP
                                for nn in (int(nl[b]), int(nh[b]))
                                for cc in _chunks(nn))
                    tri = 0
                    for half in range(2):
                        ncols_all = int(nl[b]) if half == 0 else int(nh[b])
                        col0 = int(meta["col_lo"][b]) if half == 0 \
                            else int(meta["col_hi"][b])
                        view = tbl_lo if half == 0 else tbl_hi
                        for s0 in range(0, ncols_all, MAXC):
                            ncc = min(MAXC, ncols_all - s0)
                            nc3 = ((ncc + GRP - 1) // GRP) * GRP
                            g = gp.tile([128, MAXC, RW], bf16, tag="g")
                            nc.gpsimd.dma_gather(
                                g[:, 0:ncc, :], view,
                                gidx_sb[:, (col0 + s0) * 8:(col0 + s0 + ncc) * 8],
                                ncc * 128, ncc * 128, RW,
                                single_packet=(ncc * 128 <= 1008),
                                queue_num=qctr[0] % NSWQ)
                            qctr[0] += 1
                            gh = g[:, 0:ncc, 0:128]               # [*,nc,128]
                            al = ep.tile([128, MAXC, H], f32, tag="al")
                            nc.vector.tensor_tensor(
                                out=al[:, 0:ncc, :], in0=g[:, 0:ncc, 128:132],
                                in1=adl[:].unsqueeze(1).to_broadcast(
                                    [128, ncc, H]),
                                op=mybir.AluOpType.add)
                            alf = al[:, 0:ncc, :].rearrange("p n h -> p (n h)")
                            e2 = ep.tile([128, MAXC, H], f32, tag="e2")
                            e2f = e2[:, 0:ncc, :].rearrange("p n h -> p (n h)")
                            nc.vector.tensor_scalar(
                                out=e2f, in0=alf, scalar1=NEG, scalar2=None,
                                op0=mybir.AluOpType.mult)
                            nc.vector.tensor_tensor(
                                out=e2f, in0=e2f, in1=alf,
                                op=mybir.AluOpType.max)
                            rhs = rp.tile([128, NC3, 132], bf16, tag="rhs")
                            if nc3 > ncc:
                                nc.vector.memset(rhs[:, ncc:nc3, :], 0.0)
                            nc.scalar.activation(
                                out=rhs[:, 0:ncc, 0:4], in_=e2[:, 0:ncc, :],
                                func=mybir.ActivationFunctionType.Exp)
                            nc.vector.tensor_tensor(
                                out=rhs[:, 0:ncc, 4:132].rearrange(
                                    "p n (h f) -> p n h f", f=F),
                                in0=gh.rearrange(
                                    "p n (h f) -> p n h f", f=F),
                                in1=rhs[:, 0:ncc, 0:4].unsqueeze(3).to_broadcast(
                                    [128, ncc, H, F]),
                                op=mybir.AluOpType.mult)
                            for t in range(nc3 // GRP):
                                nc.tensor.matmul(
                                    out=psum[:],
                                    lhsT=identb_sb[:],
                                    rhs=rhs[:, t * GRP:(t + 1) * GRP, :].rearrange(
                                        "p a b -> p (a b)"),
                                    start=(tri == 0), stop=(tri == n_tri - 1))
                                tri += 1
                    assert tri == n_tri

                    # ---- epilogue ----
                    U = ep.tile([128, 132], f32, tag="U")
                    nc.vector.tensor_tensor(out=U[:], in0=rhs_s[:],
                                            in1=psum[:, 0:132],
                                            op=mybir.AluOpType.add)
                    nc.vector.tensor_tensor(out=U[:], in0=U[:],
                                            in1=psum[:, 132:264],
                                            op=mybir.AluOpType.add)
                    nc.vector.tensor_tensor(out=U[:], in0=U[:],
                                            in1=psum[:, 264:396],
                                            op=mybir.AluOpType.add)
                    sden = ep.tile([128, H], f32, tag="sden")
                    nc.vector.tensor_scalar(out=sden[:], in0=U[:, 0:4],
                                            scalar1=1e-16, scalar2=None,
                                            op0=mybir.AluOpType.add)
                    rv = ep.tile([128, H], f32, tag="rv")
                    nc.vector.reciprocal(out=rv[:], in_=sden[:])
                    nc.vector.tensor_scalar(out=rv[:], in0=rv[:], scalar1=1.0 / H,
                                            scalar2=None,
                                            op0=mybir.AluOpType.mult)
                    m = ep.tile([128, H * F], f32, tag="m")
                    nc.vector.tensor_tensor(
                        out=m[:].rearrange("p (h f) -> p h f", f=F),
                        in0=U[:, 4:132].rearrange("p (h f) -> p h f", f=F),
                        in1=rv[:].unsqueeze(2).to_broadcast([128, H, F]),
                        op=mybir.AluOpType.mult)
                    o = ep.tile([128, F], f32, tag="o")
                    nc.vector.tensor_tensor(out=o[:], in0=m[:, 0:F],
                                            in1=m[:, F:2 * F],
                                            op=mybir.AluOpType.add)
                    o2 = ep.tile([128, F], f32, tag="o2t")
                    nc.vector.tensor_tensor(out=o2[:], in0=m[:, 2 * F:3 * F],
                                            in1=m[:, 3 * F:4 * F],
                                            op=mybir.AluOpType.add)
                    nc.vector.tensor_tensor(out=o[:], in0=o[:], in1=o2[:],
                                            op=mybir.AluOpType.add)
                    nc.vector.tensor_tensor(out=o[:], in0=o[:], in1=bias_sb[:],
                                            op=mybir.AluOpType.add)
                    if is_layer1:
                        # ELU
                        m0 = ep.tile([128, F], f32, tag="m0")
                        nc.vector.tensor_scalar(out=m0[:], in0=o[:], scalar1=0.0,
                                                scalar2=None,
                                                op0=mybir.AluOpType.min)
                        em = ep.tile([128, F], f32, tag="em")
                        nc.scalar.activation(out=em[:], in_=m0[:],
                                             func=mybir.ActivationFunctionType.Exp)
                        nc.vector.tensor_scalar(out=em[:], in0=em[:], scalar1=-1.0,
                                                scalar2=None,
                                                op0=mybir.AluOpType.add)
                        nc.vector.tensor_tensor(out=o[:], in0=o[:], in1=em[:],
                                                op=mybir.AluOpType.max)
                        # transpose -> o1T (bf16) + h2 = o1 @ W2 -> Ts2
                        pT = psT.tile([F, 128], f32, tag="pT")
                        nc.tensor.transpose(out=pT[:], in_=o[:],
                                            identity=identf_sb[:])
                        oT = ep.tile([F, 128], bf16, tag="oT")
                        nc.vector.tensor_copy(out=oT[:], in_=pT[:])
                        if b * 128 < SPLIT:
                            nc.sync.dma_start(
                                out=o1Ta[:, b * 128:b * 128 + w_b],
                                in_=oT[:, 0:w_b])
                        else:
                            nc.sync.dma_start(
                                out=o1Tb[:, b * 128 - SPLIT:
                                         b * 128 - SPLIT + w_b],
                                in_=oT[:, 0:w_b])
                        if b == SPLIT // 128 - 1:
                            # first-half allgather + stage-A2 overlap the
                            # remaining layer-1 blocks
                            nc.gpsimd.collective_compute(
                                "AllGather", mybir.AluOpType.bypass,
                                replica_groups=[list(range(NCORES))],
                                ins=[o1Ta[:].opt()], outs=[o1Tga[:].opt()])
                            viewsA = []
                            for r in range(NCORES):
                                for p0 in range(0, SPLIT, SLA):
                                    viewsA.append(
                                        (o1Tga[r * F:(r + 1) * F,
                                               p0:min(p0 + SLA, SPLIT)],
                                         r * SLAB + p0))
                            _stage_a(nc, (sbA, psA), viewsA, w2_sb, T2, F)
                        ps2 = psT.tile([128, 136], f32, tag="ps2")
                        nc.tensor.matmul(out=ps2[:], lhsT=oT[:], rhs=w2_sb[:],
                                         start=True, stop=True)
                        st2 = ep.tile([128, RW], bf16, tag="st2")
                        nc.vector.tensor_copy(out=st2[:, 0:136],
                                              in_=ps2[:, 0:136])
                        nc.sync.dma_start(out=Ts2[b * 128:b * 128 + w_b, :],
                                          in_=st2[0:w_b, :])
                    else:
                        nc.sync.dma_start(out=out2p[b * 128:b * 128 + w_b, :],
                                          in_=o[0:w_b, :])

            # ---- stage A, layer 1: self slab first, then full table ----
            viewsS = [(xTs[:, s0:min(s0 + SLA, PERC)], s0)
                      for s0 in range(0, PERC, SLA)]
            _stage_a(nc, (sbA, psA), viewsS, w1_sb, Ts1, IN)
            views1 = [(xTg[:, s0:min(s0 + SLA, TROWS)], s0)
                      for s0 in range(0, TROWS, SLA)]
            _stage_a(nc, (sbA, psA), views1, w1_sb, T1, IN)
            nc.sync.dma_start(out=T1[SENT_ROW:SENT_ROW + 1, :], in_=sent1_sb[:])

            # ---- layer 1 edges (D-phase inlined per block: the stage-A1
            # window is DVE-bound, the edge phase has DVE headroom) ----
            edge_layer(T1, None, b1r_sb, True, tself=Ts1)

            # ---- second-half allgather; layer-2 D-phase fills the window ----
            nc.gpsimd.collective_compute(
                "AllGather", mybir.AluOpType.bypass,
                replica_groups=[list(range(NCORES))],
                ins=[o1Tb[:].opt()], outs=[o1Tgb[:].opt()])
            pre2 = d_phase(Ts2)

            # ---- stage A, layer 2 second half ----
            views2 = []
            for r in range(NCORES):
                for p0 in range(SPLIT, PERC, SLA):
                    views2.append(
                        (o1Tgb[r * F:(r + 1) * F,
                               p0 - SPLIT:min(p0 + SLA, PERC) - SPLIT],
                         r * SLAB + p0))
            _stage_a(nc, (sbA, psA), views2, w2_sb, T2, F)
            nc.sync.dma_start(out=T2[SENT_ROW:SENT_ROW + 1, :], in_=sent2_sb[:])

            # ---- layer 2 edges ----
            edge_layer(T2, pre2, b2r_sb, False)

    nc.compile()
    return nc


_CACHE = {}


def _prepare(x, edge_index, W1, att_src1, att_dst1, b1, W2, att_src2,
             att_dst2, b2):
    x = np.asarray(x, np.float32)
    edge_index = np.asarray(edge_index, np.int64)
    key = hash(edge_index.tobytes())
    if key in _CACHE:
        meta, nc = _CACHE[key]
    else:
        meta = _preprocess(edge_index)
        nc = _build_program(meta)
        _CACHE[key] = (meta, nc)

    bf = ml_dtypes.bfloat16

    def waug(W, a_s, a_d):
        # cols 0:128 = W (h-major native); 128:132 = W@a_src; 132:136 = W@a_dst
        W = np.asarray(W, np.float32)
        ws = np.einsum("ihf,hf->ih", W.reshape(-1, H, F),
                       np.asarray(a_s, np.float32))
        wd = np.einsum("ihf,hf->ih", W.reshape(-1, H, F),
                       np.asarray(a_d, np.float32))
        return np.concatenate([W, ws, wd], axis=1).astype(bf)

    # sentinel row: asrc = -350 so exp(LRelu(asrc + adst)) ~ 0; h = 0
    sent = np.zeros((1, RW), bf)
    sent[0, 128:132] = -350.0

    # x columns in g-order (junk cols zero)
    xb = x.astype(bf)
    arr = np.zeros((TROWS, IN), bf)
    arr[meta["row_of"]] = xb
    xTg = np.ascontiguousarray(arr.T)

    common = dict(
        xTg=xTg, w1=waug(W1, att_src1, att_dst1), w2=waug(W2, att_src2, att_dst2),
        identb=np.eye(128, dtype=bf), identf=np.eye(128, dtype=np.float32),
        sent1=sent, sent2=sent,
        b1r=np.broadcast_to(np.asarray(b1, np.float32), (128, F)).copy(),
        b2r=np.broadcast_to(np.asarray(b2, np.float32), (128, F)).copy(),
    )
    in_maps = []
    for c in range(NCORES):
        xTs = np.ascontiguousarray(xb[meta["perms"][c]].T)
        in_maps.append(dict(common, gidx=meta["gidx"][c], xTs=xTs))
    return nc, in_maps, meta


def _assemble(meta, results):
    out = np.empty((N, F), np.float32)
    for c in range(NCORES):
        out[meta["perms"][c]] = results[c]["out2p"]
    return out


def kernel(**inputs):
    nc, in_maps, meta = _prepare(**inputs)
    res = run_bass_kernel_spmd(nc, in_maps, core_ids=list(range(NCORES)))
    return _assemble(meta, res.results)


def run_traced(**inputs):
    """Profiled run; returns BassKernelResults (exec_time_ns etc.)."""
    nc, in_maps, meta = _prepare(**inputs)
    res = run_bass_kernel_spmd(nc, in_maps, core_ids=list(range(NCORES)),
                               trace=True)
    res.gat_output = _assemble(meta, res.results)
    return res


# revision 47
# speedup vs baseline: 1.2233x; 1.0109x over previous
"""2-layer GAT (PyG GATConv, concat=False, self-loops) on 8 Trainium2 cores.

Design: nodes/edges partitioned by destination across 8 cores; each core
owns a 6250-dst range, dsts sorted by (degree//4, must-lo count) into 49
blocks of 128 PSUM lanes.  Per-edge source data is fetched with dma_gather
from a 512B-per-node table in HBM; each all-bf16 row is the raw psum of
one augmented matmul x @ [W | W@a_src | W@a_dst]: [h x128 | asrc x4 |
adst x4 | pad].  The gather is SWDGE descriptor-emission bound (~8ns/row
on the Q7), so row bytes are nearly free while row COUNT matters: padding
is minimized by a per-lane-balanced split of edges between two overlapping
int16-indexable views (lo=[0,32767), hi=[17241,50008) of the 50008-row
g-space table - 8 slabs of 6251 rows, one junk/sentinel row per slab), by
excluding self-loop edges (each core streams its own dst rows sequentially,
which also provides a_dst and the self contribution), and by one shared
index stream for both layers.  Padding slots point at the sentinel row
(asrc=-350 => exp(LRelu(...)) ~ 0, h=0).  Scatter is free: slot columns
accumulate into per-block PSUM via identity-weight matmuls, 3 columns per
matmul.  Gather calls batch 15 columns (two calls fit the enlarged 64KB
SWDGE descriptor ring, overlapping emission with drain; multi-packet mode
lifts the 1008-index single-packet limit).  Layer boundary: the layer-1
epilogue emits a transposed bf16 output in two column halves; each half is
AllGathered separately (the first mid-layer) and a replicated stage-A
rebuilds the layer-2 table, overlapping layer-1's tail.  Per-block dst
reads (D-phase) are precomputed during idle stage-A windows.
"""
import sys
sys.path.insert(0, "/opt/trn_rl_repo")

import numpy as np
import ml_dtypes

import concourse.bass as bass
import concourse.bacc as bacc
import concourse.mybir as mybir
from concourse.bass_utils import run_bass_kernel_spmd
from concourse.tile import TileContext

N = 50000
E = 1600000
IN = 128
H = 4
F = 32
NEG = 0.2
NCORES = 8
PERC = N // NCORES           # 6250
SLAB = PERC + 1              # 6251 (junk row at pos 6250)
TROWS = NCORES * SLAB        # 50008
NBLK = (PERC + 127) // 128   # 49
LO_END = 32767               # lo view = rows [0, 32767)
HI_START = TROWS - 32767     # 17241; hi view = rows [17241, 50008)
SENT_ROW = 3 * SLAB + PERC   # 25003 (core 3's junk row; inside the overlap)
RW = 256                     # table row: 256 bf16 slots = 512B:
                             # [h x128 | asrc x4 | adst x4 | 120 pad]
MAXC = int(__import__('os').environ.get('GAT_MAXC', '15'))  # max gather columns per dma_gather call
GRP = 3                      # aggregation matmul: 3 slot-columns per matmul


def _chunks(n):
    return [min(MAXC, n - s) for s in range(0, n, MAXC)]


def _pack_idx(idx_flat):
    """[n] -> [128, n/16] int16; idx i -> (partition i%16, col i//16), x8."""
    n = idx_flat.shape[0]
    assert n % 16 == 0
    a = idx_flat.reshape(n // 16, 16).T.astype(np.int16)
    return np.ascontiguousarray(np.tile(a, (8, 1)))


def _preprocess(edge_index):
    """Index preprocessing. Returns per-core gidx + shared structure."""
    src0 = edge_index[0].astype(np.int64)
    dst0 = edge_index[1].astype(np.int64)
    deg = np.bincount(dst0, minlength=N)          # gather-degree (no self)

    def build_perms(lo_cnt):
        perms = []
        pos_of = np.empty(N, np.int64)
        for c in range(NCORES):
            ids = np.arange(c * PERC, (c + 1) * PERC)
            order = np.lexsort((-lo_cnt[ids], -(deg[ids] // 4)))
            perm = ids[order]
            perms.append(perm)
            pos_of[perm] = np.arange(PERC)
        return perms, pos_of

    # pass 1: degree only -> provisional rows -> per-dst must-lo counts
    perms, pos_of = build_perms(np.zeros(N, np.int64))
    for _ in range(2):
        row_of = (np.arange(N) // PERC) * SLAB + pos_of
        must_lo = row_of[src0] < HI_START
        lo_cnt = np.bincount(dst0[must_lo], minlength=N)
        perms, pos_of = build_perms(lo_cnt)

    row_of = (np.arange(N) // PERC) * SLAB + pos_of
    r_src = row_of[src0]
    kind = np.where(r_src < HI_START, 0, np.where(r_src >= LO_END, 1, 2))

    lane_all = pos_of[dst0]
    core_all = dst0 // PERC
    blk_all = lane_all // 128

    # per-block slot counts nl/nh (cross-core maxima)
    A = np.zeros(NBLK, np.int64)
    B = np.zeros(NBLK, np.int64)
    C = np.zeros(NBLK, np.int64)
    cnt_l = np.zeros((NCORES, PERC), np.int64)
    cnt_h = np.zeros((NCORES, PERC), np.int64)
    cnt_f = np.zeros((NCORES, PERC), np.int64)
    for c in range(NCORES):
        m = core_all == c
        lane = lane_all[m]
        k = kind[m]
        cnt_l[c] = np.bincount(lane[k == 0], minlength=PERC)
        cnt_h[c] = np.bincount(lane[k == 1], minlength=PERC)
        cnt_f[c] = np.bincount(lane[k == 2], minlength=PERC)
    for b in range(NBLK):
        sl = slice(b * 128, min((b + 1) * 128, PERC))
        A[b] = cnt_l[:, sl].max()
        B[b] = cnt_h[:, sl].max()
        C[b] = (cnt_l[:, sl] + cnt_h[:, sl] + cnt_f[:, sl]).max()
    nl = A.copy()
    nh = np.maximum(B, C - A)
    S = int((nl + nh).sum())
    col_lo = np.zeros(NBLK, np.int64)   # column start of block's lo run
    col_hi = np.zeros(NBLK, np.int64)
    col = 0
    for b in range(NBLK):
        col_lo[b] = col
        col += nl[b]
        col_hi[b] = col
        col += nh[b]
    assert col == S

    # per-core column fill
    gidx = []
    sent_lo, sent_hi = SENT_ROW, SENT_ROW - HI_START
    for c in range(NCORES):
        m = core_all == c
        lane = lane_all[m]
        k = kind[m].copy()
        rows = r_src[m]
        # flex -> lo for the first a_i of each lane, rest -> hi
        nh_of_lane = nh[np.arange(PERC) // 128]
        a_i = np.maximum(0, cnt_h[c] + cnt_f[c] - nh_of_lane)
        # order edges by (lane, kind-rank, row); flex edges get rank by
        # position so the first a_i go to lo
        order = np.lexsort((rows, k, lane))
        lane_s, k_s, rows_s = lane[order], k[order], rows[order]
        # cumcount within (lane, kind)
        start = np.r_[True, (lane_s[1:] != lane_s[:-1]) | (k_s[1:] != k_s[:-1])]
        grp_id = np.cumsum(start) - 1
        first_pos = np.full(grp_id[-1] + 1, 1 << 62, np.int64)
        np.minimum.at(first_pos, grp_id, np.arange(len(grp_id)))
        cum = np.arange(len(grp_id)) - first_pos[grp_id]
        # stream + slot per edge
        is_lo = (k_s == 0) | ((k_s == 2) & (cum < a_i[lane_s]))
        slot = np.where(
            k_s == 0, cum,                                   # must-lo
            np.where(k_s == 1, cum,                          # must-hi
                     np.where(is_lo, cnt_l[c][lane_s] + cum,  # flex->lo
                              cnt_h[c][lane_s] + cum - a_i[lane_s])))  # flex->hi
        b_s = lane_s // 128
        colidx = np.where(is_lo, col_lo[b_s] + slot, col_hi[b_s] + slot)
        grid = np.full((S, 128), -1, np.int64)
        grid[colidx, lane_s % 128] = rows_s
        # sentinels + view-relative index
        lo_cols = np.zeros(S, bool)
        for b in range(NBLK):
            lo_cols[col_lo[b]:col_lo[b] + nl[b]] = True
        grid[lo_cols] = np.where(grid[lo_cols] < 0, sent_lo, grid[lo_cols])
        grid[~lo_cols] = np.where(grid[~lo_cols] < 0, sent_hi + HI_START,
                                  grid[~lo_cols]) - HI_START
        assert grid.min() >= 0 and grid.max() < LO_END
        gidx.append(_pack_idx(grid.reshape(-1)))

    return dict(gidx=gidx, nl=nl, nh=nh, S=S, col_lo=col_lo, col_hi=col_hi,
                perms=perms, pos_of=pos_of, row_of=row_of)


SLA = 8 * 128   # stage-A slab width


def _stage_a(nc, pools, views, w_sb, tbl, kdim):
    """[h | asrc | adst] = lhsT.T @ Waug -> rows packed into tbl.

    views: list of (lhsT_view [kdim, W], row0); W <= SLA.  Row layout
    (all bf16, matching the augmented-matmul psum column order):
    [h x128 | asrc x4 | adst x4 | 120 slots garbage].
    """
    sb, ps = pools
    f32, bf16 = mybir.dt.float32, mybir.dt.bfloat16
    for (view, row0) in views:
        Wt = view.shape[1]
        nt = (Wt + 127) // 128
        xsb = sb.tile([kdim, SLA], bf16, tag="xa")
        nc.sync.dma_start(out=xsb[:, 0:Wt], in_=view)
        stg = sb.tile([128, SLA // 128, RW], bf16, tag="sa")
        for t in range(nt):
            w = min(128, Wt - t * 128)
            psum = ps.tile([128, 136], f32, tag="pa")
            nc.tensor.matmul(out=psum[0:w, :], lhsT=xsb[:, t * 128:t * 128 + w],
                             rhs=w_sb[:], start=True, stop=True)
            nc.vector.tensor_copy(out=stg[0:w, t, 0:136], in_=psum[0:w, 0:136])
        nfull = Wt // 128
        if nfull:
            nc.sync.dma_start(
                out=tbl[row0:row0 + nfull * 128, :].rearrange(
                    "(t p) c -> p t c", p=128),
                in_=stg[:, 0:nfull, :])
        if Wt % 128:
            nc.sync.dma_start(out=tbl[row0 + nfull * 128:row0 + Wt, :],
                              in_=stg[0:Wt % 128, nfull, :])


def _build_program(meta):
    nl, nh, S = meta["nl"], meta["nh"], meta["S"]
    import os as _os
    NSWQ = int(_os.environ.get('GAT_NSWQ', '1'))
    nc = bacc.Bacc("TRN2", target_bir_lowering=False, debug=False,
                   num_devices=NCORES, num_swdge_queues=NSWQ,
                   dynamic_dma_scratch_size=int(_os.environ.get('GAT_SCRATCH', '65536')))

    f32, bf16, i16 = mybir.dt.float32, mybir.dt.bfloat16, mybir.dt.int16
    xTg = nc.dram_tensor("xTg", [IN, TROWS], bf16, kind="ExternalInput")
    xTs = nc.dram_tensor("xTs", [IN, PERC], bf16, kind="ExternalInput")
    w1 = nc.dram_tensor("w1", [IN, 136], bf16, kind="ExternalInput")
    w2 = nc.dram_tensor("w2", [F, 136], bf16, kind="ExternalInput")
    gidx = nc.dram_tensor("gidx", [128, S * 8], i16, kind="ExternalInput")
    identb = nc.dram_tensor("identb", [128, 128], bf16, kind="ExternalInput")
    identf = nc.dram_tensor("identf", [128, 128], f32, kind="ExternalInput")
    sent1 = nc.dram_tensor("sent1", [1, RW], bf16, kind="ExternalInput")
    sent2 = nc.dram_tensor("sent2", [1, RW], bf16, kind="ExternalInput")
    b1r = nc.dram_tensor("b1r", [128, F], f32, kind="ExternalInput")
    b2r = nc.dram_tensor("b2r", [128, F], f32, kind="ExternalInput")

    T1 = nc.dram_tensor("T1", [TROWS, RW], bf16)
    T2 = nc.dram_tensor("T2", [TROWS, RW], bf16)
    Ts1 = nc.dram_tensor("Ts1", [PERC, RW], bf16)
    Ts2 = nc.dram_tensor("Ts2", [PERC, RW], bf16)
    SPLIT = 41 * 128             # o1T column split (block 41 boundary)
    o1Ta = nc.dram_tensor("o1Ta", [F, SPLIT], bf16)
    o1Tb = nc.dram_tensor("o1Tb", [F, PERC - SPLIT], bf16)
    o1Tga = nc.dram_tensor("o1Tga", [NCORES * F, SPLIT], bf16,
                           addr_space="Shared")
    o1Tgb = nc.dram_tensor("o1Tgb", [NCORES * F, PERC - SPLIT], bf16,
                           addr_space="Shared")
    out2p = nc.dram_tensor("out2p", [PERC, F], f32, kind="ExternalOutput")

    NC3 = ((MAXC + GRP - 1) // GRP) * GRP  # rhs tile columns (33)

    with TileContext(nc) as tc:
        with (
            tc.tile_pool(name="cons", bufs=1) as cons,
            tc.tile_pool(name="sbA", bufs=3) as sbA,
            tc.tile_pool(name="psA", bufs=2, space="PSUM") as psA,
            tc.tile_pool(name="dp", bufs=2) as dp,
            tc.tile_pool(name="gp", bufs=4) as gp,
            tc.tile_pool(name="rp", bufs=3) as rp,
            tc.tile_pool(name="ep", bufs=8) as ep,
            tc.tile_pool(name="pp", bufs=1) as pp,
            tc.tile_pool(name="psE", bufs=3, space="PSUM") as psE,
            tc.tile_pool(name="psT", bufs=1, space="PSUM") as psT,
        ):
            identb_sb = cons.tile([128, 128], bf16)
            nc.sync.dma_start(out=identb_sb[:], in_=identb[:, :])
            identf_sb = cons.tile([128, 128], f32)
            nc.sync.dma_start(out=identf_sb[:], in_=identf[:, :])
            w1_sb = cons.tile([IN, 136], bf16)
            nc.sync.dma_start(out=w1_sb[:], in_=w1[:, :])
            w2_sb = cons.tile([F, 136], bf16)
            nc.sync.dma_start(out=w2_sb[:], in_=w2[:, :])
            b1r_sb = cons.tile([128, F], f32)
            nc.sync.dma_start(out=b1r_sb[:], in_=b1r[:, :])
            b2r_sb = cons.tile([128, F], f32)
            nc.sync.dma_start(out=b2r_sb[:], in_=b2r[:, :])
            sent1_sb = cons.tile([1, RW], bf16)
            nc.sync.dma_start(out=sent1_sb[:], in_=sent1[:, :])
            sent2_sb = cons.tile([1, RW], bf16)
            nc.sync.dma_start(out=sent2_sb[:], in_=sent2[:, :])
            gidx_sb = cons.tile([128, S * 8], i16)
            nc.sync.dma_start(out=gidx_sb[:], in_=gidx[:, :])

            def d_block(tself, b):
                """One block's dst-slab read + self-edge prep; returns
                (adl, rhs_s) tiles that persist until the epilogue."""
                if True:
                    w_b = min(128, PERC - b * 128)
                    dt = dp.tile([128, RW], bf16, tag="dt")
                    if w_b < 128:
                        nc.vector.memset(dt[:], 0.0)
                    nc.sync.dma_start(out=dt[0:w_b, :],
                                      in_=tself[b * 128:b * 128 + w_b, :])
                    dhb = dt[:, 0:128]                 # [128, 128] h-major
                    adl = pp.tile([128, H], bf16, tag=f"adl{b}")
                    nc.vector.tensor_copy(out=adl[:], in_=dt[:, 132:136])
                    # self edge: e = LRelu(asrc + adst); w = exp(e)
                    es = ep.tile([128, H], f32, tag="es")
                    nc.vector.tensor_tensor(out=es[:], in0=dt[:, 128:132],
                                            in1=dt[:, 132:136],
                                            op=mybir.AluOpType.add)
                    es2 = ep.tile([128, H], f32, tag="es2")
                    nc.vector.tensor_scalar(out=es2[:], in0=es[:], scalar1=NEG,
                                            scalar2=None, op0=mybir.AluOpType.mult)
                    nc.vector.tensor_tensor(out=es2[:], in0=es2[:], in1=es[:],
                                            op=mybir.AluOpType.max)
                    wsb = ep.tile([128, H], bf16, tag="wsb")
                    nc.scalar.activation(out=wsb[:], in_=es2[:],
                                         func=mybir.ActivationFunctionType.Exp)
                    rhs_s = pp.tile([128, 4 + H * F], f32, tag=f"rs{b}")
                    nc.vector.tensor_copy(out=rhs_s[:, 0:4], in_=wsb[:])
                    nc.vector.tensor_tensor(
                        out=rhs_s[:, 4:132].rearrange("p (h f) -> p h f", f=F),
                        in0=dhb.rearrange("p (h f) -> p h f", f=F),
                        in1=wsb[:].unsqueeze(2).to_broadcast([128, H, F]),
                        op=mybir.AluOpType.mult)
                    return (adl, rhs_s)

            def d_phase(tself):
                return {b: d_block(tself, b) for b in range(NBLK)}

            qctr = [0]

            def edge_layer(tbl, pre, bias_sb, is_layer1, tself=None):
                tbl_lo = tbl[0:LO_END, :]
                tbl_hi = tbl[HI_START:TROWS, :]
                for b in range(NBLK):
                    w_b = min(128, PERC - b * 128)
                    adl, rhs_s = pre[b] if pre is not None else \
                        d_block(tself, b)

                    # ---- gathered slots ----
                    psum = psE.tile([128, GRP * 132], f32, tag="acc")
                    n_tri = sum((cc + GRP - 1) // GRP
                                for nn in (int(nl[b]), int(nh[b]))
                                for cc in _chunks(nn))
                    tri = 0
                    for half in range(2):
                        ncols_all = int(nl[b]) if half == 0 else int(nh[b])
                        col0 = int(meta["col_lo"][b]) if half == 0 \
                            else int(meta["col_hi"][b])
                        view = tbl_lo if half == 0 else tbl_hi
                        for s0 in range(0, ncols_all, MAXC):
                            ncc = min(MAXC, ncols_all - s0)
                            nc3 = ((ncc + GRP - 1) // GRP) * GRP
                            g = gp.tile([128, MAXC, RW], bf16, tag="g")
                            nc.gpsimd.dma_gather(
                                g[:, 0:ncc, :], view,
                                gidx_sb[:, (col0 + s0) * 8:(col0 + s0 + ncc) * 8],
                                ncc * 128, ncc * 128, RW,
                                single_packet=(ncc * 128 <= 1008),
                                queue_num=qctr[0] % NSWQ)
                            qctr[0] += 1
                            gh = g[:, 0:ncc, 0:128]               # [*,nc,128]
                            al = ep.tile([128, MAXC, H], f32, tag="al")
                            nc.vector.tensor_tensor(
                                out=al[:, 0:ncc, :], in0=g[:, 0:ncc, 128:132],
                                in1=adl[:].unsqueeze(1).to_broadcast(
                                    [128, ncc, H]),
                                op=mybir.AluOpType.add)
                            alf = al[:, 0:ncc, :].rearrange("p n h -> p (n h)")
                            e2 = ep.tile([128, MAXC, H], f32, tag="e2")
                            e2f = e2[:, 0:ncc, :].rearrange("p n h -> p (n h)")
                            nc.vector.tensor_scalar(
                                out=e2f, in0=alf, scalar1=NEG, scalar2=None,
                                op0=mybir.AluOpType.mult)
                            nc.vector.tensor_tensor(
                                out=e2f, in0=e2f, in1=alf,
                                op=mybir.AluOpType.max)
                            rhs = rp.tile([128, NC3, 132], bf16, tag="rhs")
                            if nc3 > ncc:
                                nc.vector.memset(rhs[:, ncc:nc3, :], 0.0)
                            nc.scalar.activation(
                                out=rhs[:, 0:ncc, 0:4], in_=e2[:, 0:ncc, :],
                                func=mybir.ActivationFunctionType.Exp)
                            nc.vector.tensor_tensor(
                                out=rhs[:, 0:ncc, 4:132].rearrange(
                                    "p n (h f) -> p n h f", f=F),
                                in0=gh.rearrange(
                                    "p n (h f) -> p n h f", f=F),
                                in1=rhs[:, 0:ncc, 0:4].unsqueeze(3).to_broadcast(
                                    [128, ncc, H, F]),
                                op=mybir.AluOpType.mult)
                            for t in range(nc3 // GRP):
                                nc.tensor.matmul(
                                    out=psum[:],
                                    lhsT=identb_sb[:],
                                    rhs=rhs[:, t * GRP:(t + 1) * GRP, :].rearrange(
                                        "p a b -> p (a b)"),
                                    start=(tri == 0), stop=(tri == n_tri - 1))
                                tri += 1
                    assert tri == n_tri

                    # ---- epilogue ----
                    U = ep.tile([128, 132], f32, tag="U")
                    nc.vector.tensor_tensor(out=U[:], in0=rhs_s[:],
                                            in1=psum[:, 0:132],
                                            op=mybir.AluOpType.add)
                    nc.vector.tensor_tensor(out=U[:], in0=U[:],
                                            in1=psum[:, 132:264],
                                            op=mybir.AluOpType.add)
                    nc.vector.tensor_tensor(out=U[:], in0=U[:],
                                            in1=psum[:, 264:396],
                                            op=mybir.AluOpType.add)
                    sden = ep.tile([128, H], f32, tag="sden")
                    nc.vector.tensor_scalar(out=sden[:], in0=U[:, 0:4],
                                            scalar1=1e-16, scalar2=None,
                                            op0=mybir.AluOpType.add)
                    rv = ep.tile([128, H], f32, tag="rv")
                    nc.vector.reciprocal(out=rv[:], in_=sden[:])
                    nc.vector.tensor_scalar(out=rv[:], in0=rv[:], scalar1=1.0 / H,
                                            scalar2=None,
                                            op0=mybir.AluOpType.mult)
                    m = ep.tile([128, H * F], f32, tag="m")
                    nc.vector.tensor_tensor(
                        out=m[:].rearrange("p (h f) -> p h f", f=F),
                        in0=U[:, 4:132].rearrange("p (h f) -> p h f", f=F),
                        in1=rv[:].unsqueeze(2).to_broadcast([128, H, F]),
                        op=mybir.AluOpType.mult)
                    o = ep.tile([128, F], f32, tag="o")
                    nc.vector.tensor_tensor(out=o[:], in0=m[:, 0:F],
                                            in1=m[:, F:2 * F],
                                            op=mybir.AluOpType.add)
                    o2 = ep.tile([128, F], f32, tag="o2t")
                    nc.vector.tensor_tensor(out=o2[:], in0=m[:, 2 * F:3 * F],
                                            in1=m[:, 3 * F:4 * F],
                                            op=mybir.AluOpType.add)
                    nc.vector.tensor_tensor(out=o[:], in0=o[:], in1=o2[:],
                                            op=mybir.AluOpType.add)
                    nc.vector.tensor_tensor(out=o[:], in0=o[:], in1=bias_sb[:],
                                            op=mybir.AluOpType.add)
                    if is_layer1:
                        # ELU
                        m0 = ep.tile([128, F], f32, tag="m0")
                        nc.vector.tensor_scalar(out=m0[:], in0=o[:], scalar1=0.0,
                                                scalar2=None,
                                                op0=mybir.AluOpType.min)
                        em = ep.tile([128, F], f32, tag="em")
                        nc.scalar.activation(out=em[:], in_=m0[:],
                                             func=mybir.ActivationFunctionType.Exp)
                        nc.vector.tensor_scalar(out=em[:], in0=em[:], scalar1=-1.0,
                                                scalar2=None,
                                                op0=mybir.AluOpType.add)
                        nc.vector.tensor_tensor(out=o[:], in0=o[:], in1=em[:],
                                                op=mybir.AluOpType.max)
                        # transpose -> o1T (bf16) + h2 = o1 @ W2 -> Ts2
                        pT = psT.tile([F, 128], f32, tag="pT")
                        nc.tensor.transpose(out=pT[:], in_=o[:],
                                            identity=identf_sb[:])
                        oT = ep.tile([F, 128], bf16, tag="oT")
                        nc.vector.tensor_copy(out=oT[:], in_=pT[:])
                        if b * 128 < SPLIT:
                            nc.sync.dma_start(
                                out=o1Ta[:, b * 128:b * 128 + w_b],
                                in_=oT[:, 0:w_b])
                        else:
                            nc.sync.dma_start(
                                out=o1Tb[:, b * 128 - SPLIT:
                                         b * 128 - SPLIT + w_b],
                                in_=oT[:, 0:w_b])
                        if b == SPLIT // 128 - 1:
                            # first-half allgather + stage-A2 overlap the
                            # remaining layer-1 blocks
                            nc.gpsimd.collective_compute(
                                "AllGather", mybir.AluOpType.bypass,
                                replica_groups=[list(range(NCORES))],
                                ins=[o1Ta[:].opt()], outs=[o1Tga[:].opt()])
                            viewsA = []
                            for r in range(NCORES):
                                for p0 in range(0, SPLIT, SLA):
                                    viewsA.append(
                                        (o1Tga[r * F:(r + 1) * F,
                                               p0:min(p0 + SLA, SPLIT)],
                                         r * SLAB + p0))
                            _stage_a(nc, (sbA, psA), viewsA, w2_sb, T2, F)
                        ps2 = psT.tile([128, 136], f32, tag="ps2")
                        nc.tensor.matmul(out=ps2[:], lhsT=oT[:], rhs=w2_sb[:],
                                         start=True, stop=True)
                        st2 = ep.tile([128, RW], bf16, tag="st2")
                        nc.vector.tensor_copy(out=st2[:, 0:136],
                                              in_=ps2[:, 0:136])
                        nc.sync.dma_start(out=Ts2[b * 128:b * 128 + w_b, :],
                                          in_=st2[0:w_b, :])
                    else:
                        nc.sync.dma_start(out=out2p[b * 128:b * 128 + w_b, :],
                                          in_=o[0:w_b, :])

            # ---- stage A, layer 1: self slab first, then full table ----
            viewsS = [(xTs[:, s0:min(s0 + SLA, PERC)], s0)
                      for s0 in range(0, PERC, SLA)]
            _stage_a(nc, (sbA, psA), viewsS, w1_sb, Ts1, IN)
            views1 = [(xTg[:, s0:min(s0 + SLA, TROWS)], s0)
                      for s0 in range(0, TROWS, SLA)]
            _stage_a(nc, (sbA, psA), views1, w1_sb, T1, IN)
            nc.sync.dma_start(out=T1[SENT_ROW:SENT_ROW + 1, :], in_=sent1_sb[:])

            # ---- layer 1 edges (D-phase inlined per block: the stage-A1
            # window is DVE-bound, the edge phase has DVE headroom) ----
            edge_layer(T1, None, b1r_sb, True, tself=Ts1)

            # ---- second-half allgather; layer-2 D-phase fills the window ----
            nc.gpsimd.collective_compute(
                "AllGather", mybir.AluOpType.bypass,
                replica_groups=[list(range(NCORES))],
                ins=[o1Tb[:].opt()], outs=[o1Tgb[:].opt()])
            pre2 = d_phase(Ts2)

            # ---- stage A, layer 2 second half ----
            views2 = []
            for r in range(NCORES):
                for p0 in range(SPLIT, PERC, SLA):
                    views2.append(
                        (o1Tgb[r * F:(r + 1) * F,
                               p0 - SPLIT:min(p0 + SLA, PERC) - SPLIT],
                         r * SLAB + p0))
            _stage_a(nc, (sbA, psA), views2, w2_sb, T2, F)
            nc.sync.dma_start(out=T2[SENT_ROW:SENT_ROW + 1, :], in_=sent2_sb[:])

            # ---- layer 2 edges ----
            edge_layer(T2, pre2, b2r_sb, False)

    nc.compile()
    return nc


_CACHE = {}


def _prepare(x, edge_index, W1, att_src1, att_dst1, b1, W2, att_src2,
             att_dst2, b2):
    x = np.asarray(x, np.float32)
    edge_index = np.asarray(edge_index, np.int64)
    key = hash(edge_index.tobytes())
    if key in _CACHE:
        meta, nc = _CACHE[key]
    else:
        meta = _preprocess(edge_index)
        nc = _build_program(meta)
        _CACHE[key] = (meta, nc)

    bf = ml_dtypes.bfloat16

    def waug(W, a_s, a_d):
        # cols 0:128 = W (h-major native); 128:132 = W@a_src; 132:136 = W@a_dst
        W = np.asarray(W, np.float32)
        ws = np.einsum("ihf,hf->ih", W.reshape(-1, H, F),
                       np.asarray(a_s, np.float32))
        wd = np.einsum("ihf,hf->ih", W.reshape(-1, H, F),
                       np.asarray(a_d, np.float32))
        return np.concatenate([W, ws, wd], axis=1).astype(bf)

    # sentinel row: asrc = -350 so exp(LRelu(asrc + adst)) ~ 0; h = 0
    sent = np.zeros((1, RW), bf)
    sent[0, 128:132] = -350.0

    # x columns in g-order (junk cols zero)
    xb = x.astype(bf)
    arr = np.zeros((TROWS, IN), bf)
    arr[meta["row_of"]] = xb
    xTg = np.ascontiguousarray(arr.T)

    common = dict(
        xTg=xTg, w1=waug(W1, att_src1, att_dst1), w2=waug(W2, att_src2, att_dst2),
        identb=np.eye(128, dtype=bf), identf=np.eye(128, dtype=np.float32),
        sent1=sent, sent2=sent,
        b1r=np.broadcast_to(np.asarray(b1, np.float32), (128, F)).copy(),
        b2r=np.broadcast_to(np.asarray(b2, np.float32), (128, F)).copy(),
    )
    in_maps = []
    for c in range(NCORES):
        xTs = np.ascontiguousarray(xb[meta["perms"][c]].T)
        in_maps.append(dict(common, gidx=meta["gidx"][c], xTs=xTs))
    return nc, in_maps, meta


def _assemble(meta, results):
    out = np.empty((N, F), np.float32)
    for c in range(NCORES):
        out[meta["perms"][c]] = results[c]["out2p"]
    return out


def kernel(**inputs):
    nc, in_maps, meta = _prepare(**inputs)
    res = run_bass_kernel_spmd(nc, in_maps, core_ids=list(range(NCORES)))
    return _assemble(meta, res.results)


def run_traced(**inputs):
    """Profiled run; returns BassKernelResults (exec_time_ns etc.)."""
    nc, in_maps, meta = _prepare(**inputs)
    res = run_bass_kernel_spmd(nc, in_maps, core_ids=list(range(NCORES)),
                               trace=True)
    res.gat_output = _assemble(meta, res.results)
    return res
